# revision 1
# baseline (speedup 1.0000x reference)
"""Trainium2 Bass kernel for a 2-layer xLSTM (sLSTM -> mLSTM).

Strategy (8 NeuronCores, data-parallel over batch, 1 batch element/core):
  - Layer 0 (sLSTM): serial scan over T=1024. State kept UNNORMALIZED
    (c,n scaled by exp(m)) so no per-step max-stabilizer chain is needed:
      cn = exp(ft)*cn + exp(it - mu)*[z, 1]
    Every 16 steps the state is rescaled by an exact power of two
    (exponent-field extraction) and the log-offset mu is folded into the
    future i-gate pre-activations (Gx patch). h = 0.5*(1+tanh(o/2))*c/n
    with the o-gate weights pre-halved (sigmoid via tanh).
  - Layer 1 (mLSTM): chunkwise-parallel formulation (chunk L=128). The
    per-channel gates factor as exp(a_s - u_t) with a = i_logit - cumsum(f),
    u = running max(0, a), so each chunk reduces to a handful of 128x128
    matmuls + cumulative scans. Work for chunk c is interleaved into the
    serial sLSTM steps of chunk c+1 where the engines are otherwise idle.

kernel(**inputs) takes the FULL inputs and returns the FULL (B,T,H) output.
"""

import numpy as np

import concourse.bacc as bacc
import concourse.tile as tile
from concourse import mybir
from concourse.masks import make_identity, make_upper_triangular

AF = mybir.ActivationFunctionType
OP = mybir.AluOpType
FP32 = mybir.dt.float32
U32 = mybir.dt.uint32

B, T, I, H = 8, 1024, 128, 128
L = 128
NCHUNK = T // L
RENORM = 16
LN2 = 0.6931471805599453

TRACE = False
LAST_RESULTS = None
_NC_CACHE = {}


def _emit_slstm_step(nc, st, t, psG):
    """One serial sLSTM step. h1[:, t+1] <- step(h1[:, t])."""
    gps = psG.tile([H, 4], FP32, tag="g", name="gps")
    # gate pre-activations: psum = Gx[:, t, :] + sR_g @ h
    nc.tensor.matmul(gps, st["ident"], st["Gx"][:, t, :], start=True, stop=False)
    for g in range(4):
        nc.tensor.matmul(
            gps[:, g : g + 1],
            st["sRT4"][:, g * H : (g + 1) * H],
            st["h1"][:, t : t + 1],
            start=False,
            stop=(g == 3),
        )
    # eif = exp(psum[:, i,f]); z1[:,0:2] = tanh(psum[:, z,o])
    nc.scalar.activation(st["eif"], gps[:, 0:2], AF.Exp)
    nc.scalar.activation(st["z1"][:, 0:2], gps[:, 2:4], AF.Tanh)
    # iz1 = ei * [z, 1]
    nc.vector.tensor_scalar(
        st["iz1"], st["z1"][:, 0:3:2], st["eif"][:, 0:1], None, OP.mult
    )
    # cn = ef*cn + iz1
    nc.vector.scalar_tensor_tensor(
        st["cn"], st["cn"], st["eif"][:, 1:2], st["iz1"], OP.mult, OP.add
    )
    nc.vector.reciprocal(st["rr"], st["cn"][:, 1:2])
    # cr = 0.5 * c / n
    nc.vector.tensor_scalar(
        st["cr"], st["cn"][:, 0:1], st["rr"], 0.5, OP.mult, OP.mult
    )
    # h = to*cr + cr  (= sigmoid(o)*c/n with o pre-halved)
    nc.vector.scalar_tensor_tensor(
        st["h1"][:, t + 1 : t + 2], st["z1"][:, 1:2], st["cr"], st["cr"],
        OP.mult, OP.add,
    )
    if (t + 1) % RENORM == 0:
        _emit_renorm(nc, st, t)


def _emit_renorm(nc, st, t):
    """Rescale cn by 2^-e2(n) exactly; fold ln of the scale into future Gx_i."""
    cn_u = st["cn"][:, 1:2].bitcast(U32)
    nc.vector.tensor_scalar(
        st["p2"].bitcast(U32), cn_u, 0x7F800000, None, OP.bitwise_and
    )
    nc.vector.tensor_scalar(
        st["e2"].bitcast(U32), cn_u, 23, 0x4B000000,
        OP.logical_shift_right, OP.bitwise_or,
    )
    # negdelta = -(e_biased - 127) * ln2 ; e2 holds 2^23 + e_biased as fp32
    nc.vector.tensor_scalar(
        st["nd"], st["e2"], -8388735.0, -LN2, OP.add, OP.mult
    )
    nc.vector.tensor_tensor(st["negmu"], st["negmu"], st["nd"], OP.add)
    nc.vector.reciprocal(st["rs"], st["p2"])
    nc.vector.tensor_scalar(st["cn"], st["cn"], st["rs"], None, OP.mult)
    if t + 1 < T:
        hi = min(t + 1 + RENORM, T)
        sl = st["Gx"][:, t + 1 : hi, 0:1]
        nc.vector.tensor_scalar(sl, sl, st["negmu"], None, OP.add)


def _mlstm_chunk_ops(nc, st, ci, psB, chk, hout_d):
    """Return a list of closures, each emitting one instruction of mLSTM
    chunk ci. Layouts: channel on partitions ([a, t]) except where noted."""
    s0 = ci * L
    sl = slice(s0, s0 + L)
    h1sl = slice(1 + s0, 1 + s0 + L)
    last = ci == NCHUNK - 1
    ops = []

    # -- projections q,k,v,it,ft,to (o pre-halved, k pre-scaled)
    PROJ = [("q_", AF.Identity), ("k_", AF.Identity), ("v_", AF.Identity),
            ("it_", AF.Identity), ("ft_", AF.Identity), ("tom", AF.Tanh)]

    def mk_proj(j, name, func):
        def mm():
            ps = psB.tile([H, L], FP32, tag="ps", name="proj_ps")
            chk["proj_ps"] = ps
            nc.tensor.matmul(
                ps, st["WT6"][:, j * H : (j + 1) * H], st["h1"][:, h1sl],
                start=True, stop=True,
            )
        def cp():
            nc.scalar.activation(
                st[name][:, sl], chk["proj_ps"], func,
                bias=st["b6"][:, j : j + 1],
            )
        return [mm, cp]

    for j, (name, func) in enumerate(PROJ):
        ops += mk_proj(j, name, func)

    # -- gate scans: F = cumsum(ft); a = it - F; u = runmax(0, a)
    def scan_F():
        init = 0.0 if ci == 0 else st["F_"][:, s0 - 1 : s0]
        nc.vector.tensor_tensor_scan(
            st["F_"][:, sl], st["ft_"][:, sl], st["zerL"], init, OP.add, OP.add
        )
    def calc_a():
        nc.vector.tensor_tensor(
            st["a_"][:, sl], st["it_"][:, sl], st["F_"][:, sl], OP.subtract
        )
    def scan_u():
        init = 0.0 if ci == 0 else st["u_"][:, s0 - 1 : s0]
        nc.vector.tensor_tensor_scan(
            st["u_"][:, sl], st["a_"][:, sl], st["zerL"], init, OP.max, OP.add
        )
    ops += [scan_F, calc_a, scan_u]

    u_end = st["u_"][:, s0 + L - 1 : s0 + L]

    def calc_negu():
        nc.vector.tensor_scalar(st["negu"], u_end, -1.0, None, OP.mult)
    def calc_P():
        nc.scalar.activation(st["Pc"], st["a_"][:, sl], AF.Exp, bias=st["negu"])
    def calc_E():
        nc.scalar.activation(st["Ec"], st["u_"][:, sl], AF.Exp,
                             bias=u_end, scale=-1.0)
    ops += [calc_negu, calc_P, calc_E]

    if ci > 0:
        def calc_d():
            nc.scalar.activation(st["ddec"], st["u_"][:, s0 - 1 : s0], AF.Exp,
                                 bias=st["negu"])
        def scale_Cs():
            nc.vector.tensor_scalar(st["CsS"], st["Cs"], st["ddec"], None, OP.mult)
        def tr_Cs():
            ps = psB.tile([H, H], FP32, tag="ps2", name="cst_ps")
            chk["cst_ps"] = ps
            nc.tensor.transpose(ps, st["CsS"], st["ident"])
        def cp_Cst():
            nc.vector.tensor_copy(st["Cst"], chk["cst_ps"])
        def calc_dn():
            nc.vector.tensor_scalar(
                st["dn"], st["Ncum"][:, s0 - 1 : s0], st["ddec"], None, OP.mult
            )
        ops += [calc_d, scale_Cs, tr_Cs, cp_Cst, calc_dn]

    # -- n accumulation (per-channel cumsum of P*k with decayed carry)
    def calc_PK():
        nc.vector.tensor_tensor(st["PKc"], st["Pc"], st["k_"][:, sl], OP.mult)
    def scan_N():
        init = 0.0 if ci == 0 else st["dn"]
        nc.vector.tensor_tensor_scan(
            st["Ncum"][:, sl], st["PKc"], st["zerL"], init, OP.add, OP.add
        )
    ops += [calc_PK, scan_N]

    # -- attention-style intra-chunk matmuls
    def mm_St():
        ps = psB.tile([L, L], FP32, tag="ps2", name="st_ps")
        chk["st_ps"] = ps
        nc.tensor.matmul(ps, st["k_"][:, sl], st["q_"][:, sl],
                         start=True, stop=True)
    def mask_S():
        nc.vector.tensor_tensor(st["Sm"], chk["st_ps"], st["tri"], OP.mult)
    def calc_PV():
        nc.vector.tensor_tensor(st["PVa"], st["Pc"], st["v_"][:, sl], OP.mult)
    def tr_PV():
        ps = psB.tile([H, L], FP32, tag="ps2", name="t_ps")
        chk["pvt_ps"] = ps
        nc.tensor.transpose(ps, st["PVa"], st["ident"])
    def cp_PVt():
        nc.vector.tensor_copy(st["PVt"], chk["pvt_ps"])
    ops += [mm_St, mask_S, calc_PV, tr_PV, cp_PVt]

    def mm_IH():
        ps = psB.tile([L, H], FP32, tag="ps3", name="ih_ps")
        chk["ih_ps"] = ps
        nc.tensor.matmul(ps, st["Sm"], st["PVt"], start=True, stop=(ci == 0))
    ops.append(mm_IH)
    if ci > 0:
        def mm_carry():
            nc.tensor.matmul(chk["ih_ps"], st["q_"][:, sl], st["Cst"],
                             start=False, stop=True)
        ops.append(mm_carry)

    def cp_IH():
        nc.vector.tensor_copy(st["IHs"], chk["ih_ps"])
    def tr_IH():
        ps = psB.tile([H, L], FP32, tag="ps2", name="t_ps")
        chk["iht_ps"] = ps
        nc.tensor.transpose(ps, st["IHs"], st["ident"])
    ops += [cp_IH, tr_IH]

    # -- denominator: row = sum_a E*Ncum*q ; rec = 0.5/max(|row|, 1)
    def calc_ENQ():
        nc.vector.tensor_tensor(st["ENQ"], st["Ncum"][:, sl], st["q_"][:, sl],
                                OP.mult)
    def calc_ENQ2():
        nc.vector.tensor_tensor(st["ENQ2"], st["ENQ"], st["Ec"], OP.mult)
    def mm_row():
        ps = psB.tile([1, L], FP32, tag="ps4", name="row_ps")
        chk["row_ps"] = ps
        nc.tensor.matmul(ps, st["ones1"], st["ENQ2"], start=True, stop=True)
    def calc_drow():
        nc.scalar.activation(st["drow"], chk["row_ps"], AF.Abs)
    def calc_drow2():
        nc.vector.tensor_scalar(st["drow2"], st["drow"], 1.0, 2.0,
                                OP.max, OP.mult)
    def calc_rrow():
        nc.vector.reciprocal(st["rrow"], st["drow2"])
    def bcast_r():
        nc.gpsimd.partition_broadcast(st["Rb"], st["rrow"])
    ops += [calc_ENQ, calc_ENQ2, mm_row, calc_drow, calc_drow2, calc_rrow,
            bcast_r]

    # -- output: h = (1+to) * E * IH * (0.5/den)
    def calc_EH():
        nc.vector.tensor_tensor(st["EH"], st["Ec"], chk["iht_ps"], OP.mult)
    def calc_EHR():
        nc.vector.tensor_tensor(st["EHR"], st["EH"], st["Rb"], OP.mult)
    def calc_t2():
        nc.vector.tensor_scalar(st["t2"], st["tom"][:, sl], 1.0, None, OP.add)
    def calc_h():
        nc.vector.tensor_tensor(st["houts"][:, sl], st["t2"], st["EHR"], OP.mult)
    def dma_h():
        nc.sync.dma_start(out=hout_d[:, sl], in_=st["houts"][:, sl])
    ops += [calc_EH, calc_EHR, calc_t2, calc_h, dma_h]

    # -- state update for next chunk
    if not last:
        def tr_K():
            ps = psB.tile([H, L], FP32, tag="ps2", name="t_ps")
            chk["kt_ps"] = ps
            nc.tensor.transpose(ps, st["k_"][:, sl], st["ident"])
        def cp_Kt():
            nc.vector.tensor_copy(st["Kts"], chk["kt_ps"])
        def mm_Cd():
            ps = psB.tile([H, H], FP32, tag="ps3", name="cd_ps")
            chk["cd_ps"] = ps
            nc.tensor.matmul(ps, st["PVt"], st["Kts"], start=True, stop=True)
        ops += [tr_K, cp_Kt, mm_Cd]
        if ci == 0:
            def upd_Cs():
                nc.vector.tensor_copy(st["Cs"], chk["cd_ps"])
        else:
            def upd_Cs():
                nc.vector.tensor_tensor(st["Cs"], st["CsS"], chk["cd_ps"], OP.add)
        ops.append(upd_Cs)

    return ops


def _build_body(nc, tc, dram):
    from contextlib import ExitStack

    with ExitStack() as ctx:
        const = ctx.enter_context(tc.tile_pool(name="const", bufs=1))
        psG = ctx.enter_context(tc.tile_pool(name="psG", bufs=2, space="PSUM"))
        psA = ctx.enter_context(tc.tile_pool(name="psA", bufs=2, space="PSUM"))
        psB = ctx.enter_context(tc.tile_pool(name="psB", bufs=1, space="PSUM"))

        st = {}

        def sb(name, shape, dtype=FP32):
            st[name] = const.tile(shape, dtype, tag=name, name=name)
            return st[name]

        # constants / weights
        for name, shape in [
            ("xT", [I, T]), ("sWT4", [I, 4 * H]), ("sRT4", [H, 4 * H]),
            ("sb4", [H, 4]), ("WT6", [H, 6 * H]), ("b6", [H, 6]),
        ]:
            sb(name, shape)
            nc.sync.dma_start(out=st[name], in_=dram[name][:])
        ident = sb("ident", [128, 128]); make_identity(nc, ident[:, :])
        tri = sb("tri", [L, L]); make_upper_triangular(nc, tri[:, :], val=1.0, diag=True)
        sb("zerL", [128, L]); nc.vector.memset(st["zerL"], 0.0)
        sb("ones1", [128, 1]); nc.vector.memset(st["ones1"], 1.0)

        # persistent buffers
        sb("Gx", [H, T, 4])
        sb("h1", [H, T + 1]); nc.vector.memset(st["h1"][:, 0:1], 0.0)
        for name in ["q_", "k_", "v_", "it_", "ft_", "tom", "F_", "a_", "u_",
                     "Ncum", "houts"]:
            sb(name, [H, T])
        for name in ["Cs", "CsS", "Cst"]:
            sb(name, [H, H])
        # sLSTM step state
        sb("cn", [H, 2]); nc.vector.memset(st["cn"], 0.0)
        sb("z1", [H, 3]); nc.vector.memset(st["z1"][:, 2:3], 1.0)
        for name in ["eif", "iz1"]:
            sb(name, [H, 2])
        for name in ["rr", "cr", "p2", "e2", "nd", "negmu", "rs"]:
            sb(name, [H, 1])
        nc.vector.memset(st["negmu"], 0.0)
        # mLSTM chunk scratch
        for name in ["Pc", "Ec", "PKc", "Sm", "PVa", "PVt", "IHs", "Kts",
                     "ENQ", "ENQ2", "Rb", "EH", "EHR", "t2"]:
            sb(name, [128, L])
        for name in ["negu", "ddec", "dn"]:
            sb(name, [H, 1])
        sb("drow", [1, L]); sb("drow2", [1, L]); sb("rrow", [1, L])

        # Gx precompute: Gx[:, tt, g] = sW_g @ x_t (+ sb_g)
        for g in range(4):
            for tt in range(T // 512):
                ps = psA.tile([H, 512], FP32, tag="gx", name="gx_ps")
                nc.tensor.matmul(
                    ps, st["sWT4"][:, g * H : (g + 1) * H],
                    st["xT"][:, tt * 512 : (tt + 1) * 512],
                    start=True, stop=True,
                )
                nc.scalar.activation(
                    st["Gx"][:, tt * 512 : (tt + 1) * 512, g], ps,
                    AF.Identity, bias=st["sb4"][:, g : g + 1],
                )

        # serial loop with interleaved mLSTM chunk work
        chk = {}
        pending = []
        for t in range(T):
            _emit_slstm_step(nc, st, t, psG)
            if pending:
                pending.pop(0)()
            if (t + 1) % L == 0:
                ci = (t + 1) // L - 1
                pending += _mlstm_chunk_ops(nc, st, ci, psB, chk, dram["hout"])
        while pending:
            pending.pop(0)()


def _get_nc():
    if "nc" in _NC_CACHE:
        return _NC_CACHE["nc"]
    nc = bacc.Bacc("TRN2", debug=False, num_devices=B)
    dram = {}
    for name, shape in [
        ("xT", [I, T]), ("sWT4", [I, 4 * H]), ("sRT4", [H, 4 * H]),
        ("sb4", [H, 4]), ("WT6", [H, 6 * H]), ("b6", [H, 6]),
    ]:
        dram[name] = nc.declare_dram_parameter(name, shape, FP32, isOutput=False)
    dram["hout"] = nc.declare_dram_parameter("hout", [H, T], FP32, isOutput=True)
    with tile.TileContext(nc) as tc:
        _build_body(nc, tc, dram)
    nc.compile()
    _NC_CACHE["nc"] = nc
    return nc


def _make_runner(nc):
    """Build a jitted SPMD runner for a compiled Bacc program (replicates
    bass2jax.run_bass_via_pjrt but reuses the jitted callable across calls)."""
    import jax
    from jax.sharding import Mesh, PartitionSpec
    from jax.experimental.shard_map import shard_map
    from concourse import mybir as _mb
    from concourse.bass2jax import (
        _bass_exec_p, install_neuronx_cc_hook, partition_id_tensor,
    )

    install_neuronx_cc_hook()
    partition_name = nc.partition_id_tensor.name if nc.partition_id_tensor else None
    in_names, out_names, out_avals, zero_outs = [], [], [], []
    for alloc in nc.m.functions[0].allocations:
        if not isinstance(alloc, _mb.MemoryLocationSet):
            continue
        name = alloc.memorylocations[0].name
        if alloc.kind == "ExternalInput":
            if name != partition_name:
                in_names.append(name)
        elif alloc.kind == "ExternalOutput":
            out_names.append(name)
            shape = tuple(alloc.tensor_shape)
            dtype = _mb.dt.np(alloc.dtype)
            out_avals.append(jax.core.ShapedArray(shape, dtype))
            zero_outs.append(np.zeros(shape, dtype))
    n_params = len(in_names)
    n_outs = len(out_avals)
    param_names = list(in_names)
    in_names = in_names + out_names
    if partition_name is not None:
        in_names.append(partition_name)

    def _body(*args):
        operands = list(args)
        if partition_name is not None:
            operands.append(partition_id_tensor())
        outs = _bass_exec_p.bind(
            *operands,
            out_avals=tuple(out_avals),
            in_names=tuple(in_names),
            out_names=tuple(out_names),
            lowering_input_output_aliases=(),
            sim_require_finite=True,
            sim_require_nnan=True,
            nc=nc,
        )
        return tuple(outs)

    devices = jax.devices()[:B]
    mesh = Mesh(np.asarray(devices), ("core",))
    in_specs = (PartitionSpec("core"),) * (n_params + n_outs)
    out_specs = (PartitionSpec("core"),) * n_outs
    sharded = jax.jit(
        shard_map(_body, mesh=mesh, in_specs=in_specs, out_specs=out_specs,
                  check_rep=False),
        donate_argnums=tuple(range(n_params, n_params + n_outs)),
        keep_unused=True,
    )

    def run(in_maps):
        concat_in = [
            np.concatenate([np.asarray(m[name]) for m in in_maps], axis=0)
            for name in param_names
        ]
        concat_zeros = [
            np.zeros((B * z.shape[0], *z.shape[1:]), z.dtype) for z in zero_outs
        ]
        out_arrs = sharded(*concat_in, *concat_zeros)
        out_arrs = [np.asarray(a) for a in out_arrs]
        return [
            {name: out_arrs[i].reshape(B, *out_avals[i].shape)[c]
             for i, name in enumerate(out_names)}
            for c in range(B)
        ]

    return run


def _get_runner():
    if "runner" not in _NC_CACHE:
        _NC_CACHE["runner"] = _make_runner(_get_nc())
    return _NC_CACHE["runner"]


def kernel(**inputs):
    global LAST_RESULTS
    f32 = np.float32
    x = np.ascontiguousarray(inputs["x"], dtype=f32)
    sW = np.asarray(inputs["sW"], f32); sR = np.asarray(inputs["sR"], f32)
    sb_ = np.asarray(inputs["sb"], f32)
    inv_sqrt_h = f32(1.0 / np.sqrt(H))

    sWT4 = np.ascontiguousarray(sW.T); sRT4 = np.ascontiguousarray(sR.T)
    sWT4[:, 3 * H :] *= 0.5; sRT4[:, 3 * H :] *= 0.5
    sb4 = np.ascontiguousarray(sb_.reshape(4, H).T)
    sb4[:, 3] *= 0.5

    WT = {}
    bvecs = []
    for j, wn, bn in [(0, "Wq", "bq"), (1, "Wk", "bk"), (2, "Wv", "bv"),
                      (3, "Wi", "bi"), (4, "Wf", "bf"), (5, "Wo", "bo")]:
        w = np.asarray(inputs[wn], f32).T.copy()
        b = np.asarray(inputs[bn], f32).copy()
        if wn == "Wk":
            w *= inv_sqrt_h; b = b * inv_sqrt_h
        if wn == "Wo":
            w *= 0.5; b = b * 0.5
        WT[j] = w
        bvecs.append(b)
    WT6 = np.ascontiguousarray(np.concatenate([WT[j] for j in range(6)], axis=1))
    b6 = np.ascontiguousarray(np.stack(bvecs, axis=1))

    run = _get_runner()
    in_maps = []
    for b_ in range(B):
        in_maps.append({
            "xT": np.ascontiguousarray(x[b_].T),
            "sWT4": sWT4, "sRT4": sRT4, "sb4": sb4, "WT6": WT6, "b6": b6,
        })
    results = run(in_maps)
    LAST_RESULTS = results
    out = np.empty((B, T, H), f32)
    for b_ in range(B):
        out[b_] = results[b_]["hout"].T
    return out



# revision 32
# speedup vs baseline: 1.0235x; 1.0235x over previous
"""Trainium2 Bass kernel for a 2-layer xLSTM (sLSTM -> mLSTM).

Strategy (8 NeuronCores, data-parallel over batch, 1 batch element/core):
  - Layer 0 (sLSTM): serial scan over T=1024. State kept UNNORMALIZED
    (c,n scaled by exp(m)) so no per-step max-stabilizer chain is needed:
      cn = exp(ft)*cn + exp(it - mu)*[z, 1]
    Every 16 steps the state is rescaled by an exact power of two
    (exponent-field extraction) and the log-offset mu is folded into the
    future i-gate pre-activations (Gx patch). h = 0.5*(1+tanh(o/2))*c/n
    with the o-gate weights pre-halved (sigmoid via tanh).
  - Layer 1 (mLSTM): chunkwise-parallel formulation (chunk L=128). The
    per-channel gates factor as exp(a_s - u_t) with a = i_logit - cumsum(f),
    u = running max(0, a), so each chunk reduces to a handful of 128x128
    matmuls + cumulative scans. Work for chunk c is interleaved into the
    serial sLSTM steps of chunk c+1 where the engines are otherwise idle.

kernel(**inputs) takes the FULL inputs and returns the FULL (B,T,H) output.
"""

import numpy as np

import concourse.bacc as bacc
import concourse.tile as tile
from concourse import mybir
from concourse.masks import make_identity, make_upper_triangular

AF = mybir.ActivationFunctionType
OP = mybir.AluOpType
FP32 = mybir.dt.float32
U32 = mybir.dt.uint32

B, T, I, H = 8, 1024, 128, 128
L = 128
NCHUNK = T // L
RENORM = 32
LN2 = 0.6931471805599453

TRACE = False
LAST_RESULTS = None
_NC_CACHE = {}

# step formulation: "base" (tanh+exp, V elementwise), "expv" (exp-only, V
# elementwise, no act-table switches — the shipped default), "acto" (exp/ln
# only, everything on the ACT engine)
VARIANT = "expv"
REPEAT = 1
NO_MLSTM = False    # ablation: skip layer-1 work entirely
NO_RENORM = False   # ablation: skip renorms (numerically wrong, timing only)
ML_PACE = 1         # mLSTM pending ops drained per sLSTM step
GP_OFF = False      # expv: compute n-branch (n', no1) on gpsimd
PACK_RECIP = False  # expv: single [128,2] reciprocal for rz+rr
PACK_CN = False     # expv: gate order [Z,i,f,Oe]; iz overwrites Z so the
                    # c/n update is ONE [128,2] scalar_tensor_tensor


def _emit_slstm_step(nc, st, t, psG):
    """One serial sLSTM step. h1[:, t+1] <- step(h1[:, t])."""
    gps = psG.tile([H, 4], FP32, tag="g", name="gps")
    if VARIANT == "expv2":
        # psum = sR_g @ h only; Gx folded in post-exp (EGx = exp(Gx+b))
        for g in range(4):
            nc.tensor.matmul(
                gps[:, g : g + 1],
                st["sRT4"][:, g * H : (g + 1) * H],
                st["h1"][:, t : t + 1],
                start=(g == 0),
                stop=(g == 3),
            )
    else:
        # gate pre-activations: psum = Gx[:, t, :] + sR_g @ h
        nc.tensor.matmul(gps, st["ident"], st["Gx"][:, t, :], start=True,
                         stop=False)
        for g in range(4):
            nc.tensor.matmul(
                gps[:, g : g + 1],
                st["sRT4"][:, g * H : (g + 1) * H],
                st["h1"][:, t : t + 1],
                start=False,
                stop=(g == 3),
            )
    if VARIANT == "base":
        _emit_elem_base(nc, st, t, gps)
    elif VARIANT == "expv":
        _emit_elem_expv(nc, st, t, gps)
    elif VARIANT == "expv2":
        _emit_elem_expv2(nc, st, t, gps)
    else:
        _emit_elem_acto(nc, st, t, gps)
    if (t + 1) % RENORM == 0 and not NO_RENORM:
        _emit_renorm(nc, st, t)


def _emit_elem_base(nc, st, t, gps):
    """Original: exp+tanh, elementwise on V. Weights: o-rows pre-halved."""
    # eif = exp(psum[:, i,f]); z1[:,0:2] = tanh(psum[:, z,o])
    nc.scalar.activation(st["eif"], gps[:, 0:2], AF.Exp)
    nc.scalar.activation(st["z1"][:, 0:2], gps[:, 2:4], AF.Tanh)
    # iz1 = ei * [z, 1]
    nc.vector.tensor_scalar(
        st["iz1"], st["z1"][:, 0:3:2], st["eif"][:, 0:1], None, OP.mult
    )
    # cn = ef*cn + iz1
    nc.vector.scalar_tensor_tensor(
        st["cn"], st["cn"], st["eif"][:, 1:2], st["iz1"], OP.mult, OP.add
    )
    nc.vector.reciprocal(st["rr"], st["cn"][:, 1:2])
    # cr = 0.5 * c / n
    nc.vector.tensor_scalar(
        st["cr"], st["cn"][:, 0:1], st["rr"], 0.5, OP.mult, OP.mult
    )
    # h = to*cr + cr  (= sigmoid(o)*c/n with o pre-halved)
    nc.vector.scalar_tensor_tensor(
        st["h1"][:, t + 1 : t + 2], st["z1"][:, 1:2], st["cr"], st["cr"],
        OP.mult, OP.add,
    )


def _emit_elem_expv(nc, st, t, gps):
    """Exp-only gates (no act-table switches). Weights: z-rows x2, o-rows
    negated. g4 = [i, f, Z=e^{2z}, Oe=e^{-o}]; tanh z = (Z-1)/(Z+1);
    sigmoid o = 1/(1+Oe); h = c'/(n'(1+Oe)) * ... all elementwise on V."""
    g4 = st["g4"]
    if PACK_CN:
        # gate order [Z, i, f, Oe] (weights reordered host-side); cn layout
        # [c, n]. iz overwrites the consumed Z column so [iz, i] is a
        # contiguous [128,2] pair for a single fused c/n update.
        nc.scalar.activation(g4, gps[:, 0:4], AF.Exp)
        Z_, i_, f_, Oe = (g4[:, k : k + 1] for k in range(4))
        nc.vector.tensor_scalar(st["zp1"], Z_, 1.0, None, OP.add)
        nc.vector.tensor_scalar(st["num"], Z_, i_, i_, OP.mult, OP.subtract)
        nc.vector.reciprocal(st["rz"], st["zp1"])
        nc.vector.tensor_tensor(Z_, st["num"], st["rz"], OP.mult)  # iz -> col0
        # [c,n] = f*[c,n] + [iz,i]
        nc.vector.scalar_tensor_tensor(
            st["cn"], st["cn"], f_, g4[:, 0:2], OP.mult, OP.add
        )
        n_ = st["cn"][:, 1:2]
        nc.vector.scalar_tensor_tensor(st["no1"], n_, Oe, n_, OP.mult, OP.add)
        nc.vector.reciprocal(st["rr"], st["no1"])
        nc.vector.tensor_scalar(
            st["h1"][:, t + 1 : t + 2], st["cn"][:, 0:1], st["rr"], None,
            OP.mult,
        )
        return
    nc.scalar.activation(g4, gps[:, 0:4], AF.Exp)
    i_, f_, Z_, Oe = (g4[:, k : k + 1] for k in range(4))
    c_, n_ = st["cn"][:, 0:1], st["cn"][:, 1:2]
    nb = nc.gpsimd if GP_OFF else nc.vector
    if PACK_RECIP:
        zp1, no1 = st["rc2"][:, 0:1], st["rc2"][:, 1:2]
        rz, rr = st["rz2"][:, 0:1], st["rz2"][:, 1:2]
    else:
        zp1, no1, rz, rr = st["zp1"], st["no1"], st["rz"], st["rr"]
    # branch 1: iz = i*(Z-1)/(Z+1)
    nc.vector.tensor_scalar(zp1, Z_, 1.0, None, OP.add)
    nc.vector.tensor_scalar(st["num"], Z_, i_, i_, OP.mult, OP.subtract)
    # branch 2: n' = f*n + i ; no1 = n'*(1+Oe) ; rd = 1/no1
    nb.scalar_tensor_tensor(n_, n_, f_, i_, OP.mult, OP.add)
    nb.scalar_tensor_tensor(no1, n_, Oe, n_, OP.mult, OP.add)
    if PACK_RECIP:
        nc.vector.reciprocal(st["rz2"], st["rc2"])
    else:
        nc.vector.reciprocal(rz, zp1)
        nc.vector.reciprocal(rr, no1)
    nc.vector.tensor_tensor(st["iz"], st["num"], rz, OP.mult)
    # c' = f*c + iz ; h = c' * rd
    nc.vector.scalar_tensor_tensor(c_, c_, f_, st["iz"], OP.mult, OP.add)
    nc.vector.tensor_scalar(
        st["h1"][:, t + 1 : t + 2], c_, rr, None, OP.mult
    )


def _emit_elem_expv2(nc, st, t, gps):
    """Like expv but Gx enters multiplicatively after the exp:
    gates = exp(R h) * EGx[:, t, :]. Saves one PE matmul per step."""
    nc.scalar.activation(st["g4"], gps[:, 0:4], AF.Exp)
    gm = st["gm"]
    nc.vector.tensor_tensor(gm, st["g4"], st["Gx"][:, t, :], OP.mult)
    i_, f_, Z_, Oe = (gm[:, k : k + 1] for k in range(4))
    c_, n_ = st["cn"][:, 0:1], st["cn"][:, 1:2]
    # independent-first emission for DVE pipelining
    nc.vector.tensor_scalar(st["zp1"], Z_, 1.0, None, OP.add)
    nc.vector.tensor_scalar(st["num"], Z_, i_, i_, OP.mult, OP.subtract)
    nc.vector.scalar_tensor_tensor(n_, n_, f_, i_, OP.mult, OP.add)
    nc.vector.reciprocal(st["rz"], st["zp1"])
    nc.vector.scalar_tensor_tensor(st["no1"], n_, Oe, n_, OP.mult, OP.add)
    nc.vector.tensor_tensor(st["iz"], st["num"], st["rz"], OP.mult)
    nc.vector.reciprocal(st["rr"], st["no1"])
    nc.vector.scalar_tensor_tensor(c_, c_, f_, st["iz"], OP.mult, OP.add)
    nc.vector.tensor_scalar(
        st["h1"][:, t + 1 : t + 2], c_, st["rr"], None, OP.mult
    )


def _emit_elem_acto(nc, st, t, gps):
    """Everything on the ACT engine (exp/ln table), zero V ops: bets on
    cross-engine sem latency being the dominant cost. Weights as expv."""
    g4 = st["g4"]
    nc.scalar.activation(g4, gps[:, 0:4], AF.Exp)
    i_, f_, Z_, Oe = (g4[:, k : k + 1] for k in range(4))
    c_, n_ = st["cn"][:, 0:1], st["cn"][:, 1:2]
    A = nc.scalar.activation
    A(st["lp"], Z_, AF.Ln, bias=1.0)                      # ln(Z+1)
    A(st["rz"], st["lp"], AF.Exp, scale=-1.0)             # 1/(Z+1)
    A(st["tz"], st["rz"], AF.Identity, bias=1.0, scale=-2.0)   # tanh z
    A(st["iz"], st["tz"], AF.Identity, scale=i_)          # i*tanh
    A(c_, f_, AF.Identity, scale=c_, bias=st["iz"])       # c' = f*c + iz
    A(n_, f_, AF.Identity, scale=n_, bias=i_)             # n' = f*n + i
    A(st["lo"], Oe, AF.Ln, bias=1.0)                      # ln(1+Oe)
    A(st["lnn"], n_, AF.Ln)                               # ln n'
    A(st["sd"], st["lo"], AF.Identity, bias=st["lnn"])    # ln n' + ln(1+Oe)
    A(st["rd"], st["sd"], AF.Exp, scale=-1.0)             # 1/(n'(1+Oe))
    A(st["h1"][:, t + 1 : t + 2], c_, AF.Identity, scale=st["rd"])


def _emit_renorm(nc, st, t):
    """Rescale cn by 2^-e2(n) exactly; fold the scale into future Gx_i
    (additively in log space for base/expv; multiplicatively for expv2)."""
    cn_u = st["cn"][:, 1:2].bitcast(U32)
    nc.vector.tensor_scalar(
        st["p2"].bitcast(U32), cn_u, 0x7F800000, None, OP.bitwise_and
    )
    nc.vector.reciprocal(st["rs"], st["p2"])
    icol = 1 if PACK_CN else 0
    if VARIANT == "expv2":
        nc.vector.tensor_scalar(st["cn"], st["cn"], st["rs"], None, OP.mult)
        nc.vector.tensor_tensor(st["sacc"], st["sacc"], st["rs"], OP.mult)
        if t + 1 < T:
            hi = min(t + 1 + RENORM, T)
            sl = st["Gx"][:, t + 1 : hi, icol : icol + 1]
            nc.vector.tensor_scalar(sl, sl, st["sacc"], None, OP.mult)
        return
    nc.vector.tensor_scalar(
        st["e2"].bitcast(U32), cn_u, 23, 0x4B000000,
        OP.logical_shift_right, OP.bitwise_or,
    )
    # negdelta = -(e_biased - 127) * ln2 ; e2 holds 2^23 + e_biased as fp32
    nc.vector.tensor_scalar(
        st["nd"], st["e2"], -8388735.0, -LN2, OP.add, OP.mult
    )
    nc.vector.tensor_tensor(st["negmu"], st["negmu"], st["nd"], OP.add)
    nc.vector.tensor_scalar(st["cn"], st["cn"], st["rs"], None, OP.mult)
    if t + 1 < T:
        hi = min(t + 1 + RENORM, T)
        sl = st["Gx"][:, t + 1 : hi, icol : icol + 1]
        nc.vector.tensor_scalar(sl, sl, st["negmu"], None, OP.add)


def _mlstm_chunk_ops(nc, st, ci, psB, chk, hout_d):
    """Return a list of closures, each emitting one instruction of mLSTM
    chunk ci. Layouts: channel on partitions ([a, t]) except where noted."""
    s0 = ci * L
    sl = slice(s0, s0 + L)
    h1sl = slice(1 + s0, 1 + s0 + L)
    last = ci == NCHUNK - 1
    ops = []

    # -- projections q,k,v,it,ft,to. base: o pre-halved + Tanh (sigmoid via
    # tanh); expv/acto: o negated + Exp (sigmoid = 1/(1+e^-o), no table switch)
    o_af = AF.Tanh if VARIANT == "base" else AF.Exp
    PROJ = [("q_", AF.Identity), ("k_", AF.Identity), ("v_", AF.Identity),
            ("it_", AF.Identity), ("ft_", AF.Identity), ("tom", o_af)]

    def mk_proj(j, name, func):
        def mm():
            ps = psB.tile([H, L], FP32, tag="ps", name="proj_ps")
            chk["proj_ps"] = ps
            nc.tensor.matmul(
                ps, st["WT6"][:, j * H : (j + 1) * H], st["h1"][:, h1sl],
                start=True, stop=True,
            )
        def cp():
            nc.scalar.activation(
                st[name][:, sl], chk["proj_ps"], func,
                bias=st["b6"][:, j : j + 1],
            )
        return [mm, cp]

    for j, (name, func) in enumerate(PROJ):
        ops += mk_proj(j, name, func)

    # -- gate scans: F = cumsum(ft); a = it - F; u = runmax(0, a)
    def scan_F():
        init = 0.0 if ci == 0 else st["F_"][:, s0 - 1 : s0]
        nc.vector.tensor_tensor_scan(
            st["F_"][:, sl], st["ft_"][:, sl], st["zerL"], init, OP.add, OP.add
        )
    def calc_a():
        nc.vector.tensor_tensor(
            st["a_"][:, sl], st["it_"][:, sl], st["F_"][:, sl], OP.subtract
        )
    def scan_u():
        init = 0.0 if ci == 0 else st["u_"][:, s0 - 1 : s0]
        nc.vector.tensor_tensor_scan(
            st["u_"][:, sl], st["a_"][:, sl], st["zerL"], init, OP.max, OP.add
        )
    ops += [scan_F, calc_a, scan_u]

    u_end = st["u_"][:, s0 + L - 1 : s0 + L]

    def calc_negu():
        nc.vector.tensor_scalar(st["negu"], u_end, -1.0, None, OP.mult)
    def calc_P():
        nc.scalar.activation(st["Pc"], st["a_"][:, sl], AF.Exp, bias=st["negu"])
    def calc_E():
        nc.scalar.activation(st["Ec"], st["u_"][:, sl], AF.Exp,
                             bias=u_end, scale=-1.0)
    ops += [calc_negu, calc_P, calc_E]

    if ci > 0:
        def calc_d():
            nc.scalar.activation(st["ddec"], st["u_"][:, s0 - 1 : s0], AF.Exp,
                                 bias=st["negu"])
        def scale_Cs():
            nc.vector.tensor_scalar(st["CsS"], st["Cs"], st["ddec"], None, OP.mult)
        def tr_Cs():
            ps = psB.tile([H, H], FP32, tag="ps2", name="cst_ps")
            chk["cst_ps"] = ps
            nc.tensor.transpose(ps, st["CsS"], st["ident"])
        def cp_Cst():
            nc.vector.tensor_copy(st["Cst"], chk["cst_ps"])
        def calc_dn():
            nc.vector.tensor_scalar(
                st["dn"], st["Ncum"][:, s0 - 1 : s0], st["ddec"], None, OP.mult
            )
        ops += [calc_d, scale_Cs, tr_Cs, cp_Cst, calc_dn]

    # -- n accumulation (per-channel cumsum of P*k with decayed carry)
    def calc_PK():
        nc.vector.tensor_tensor(st["PKc"], st["Pc"], st["k_"][:, sl], OP.mult)
    def scan_N():
        init = 0.0 if ci == 0 else st["dn"]
        nc.vector.tensor_tensor_scan(
            st["Ncum"][:, sl], st["PKc"], st["zerL"], init, OP.add, OP.add
        )
    ops += [calc_PK, scan_N]

    # -- attention-style intra-chunk matmuls
    def mm_St():
        ps = psB.tile([L, L], FP32, tag="ps2", name="st_ps")
        chk["st_ps"] = ps
        nc.tensor.matmul(ps, st["k_"][:, sl], st["q_"][:, sl],
                         start=True, stop=True)
    def mask_S():
        nc.vector.tensor_tensor(st["Sm"], chk["st_ps"], st["tri"], OP.mult)
    def calc_PV():
        nc.vector.tensor_tensor(st["PVa"], st["Pc"], st["v_"][:, sl], OP.mult)
    def tr_PV():
        ps = psB.tile([H, L], FP32, tag="ps2", name="t_ps")
        chk["pvt_ps"] = ps
        nc.tensor.transpose(ps, st["PVa"], st["ident"])
    def cp_PVt():
        nc.vector.tensor_copy(st["PVt"], chk["pvt_ps"])
    ops += [mm_St, mask_S, calc_PV, tr_PV, cp_PVt]

    def mm_IH():
        ps = psB.tile([L, H], FP32, tag="ps3", name="ih_ps")
        chk["ih_ps"] = ps
        nc.tensor.matmul(ps, st["Sm"], st["PVt"], start=True, stop=(ci == 0))
    ops.append(mm_IH)
    if ci > 0:
        def mm_carry():
            nc.tensor.matmul(chk["ih_ps"], st["q_"][:, sl], st["Cst"],
                             start=False, stop=True)
        ops.append(mm_carry)

    def cp_IH():
        nc.vector.tensor_copy(st["IHs"], chk["ih_ps"])
    def tr_IH():
        ps = psB.tile([H, L], FP32, tag="ps2", name="t_ps")
        chk["iht_ps"] = ps
        nc.tensor.transpose(ps, st["IHs"], st["ident"])
    ops += [cp_IH, tr_IH]

    # -- denominator: row = sum_a E*Ncum*q ; rec = 0.5/max(|row|, 1)
    def calc_ENQ():
        nc.vector.tensor_tensor(st["ENQ"], st["Ncum"][:, sl], st["q_"][:, sl],
                                OP.mult)
    def calc_ENQ2():
        nc.vector.tensor_tensor(st["ENQ2"], st["ENQ"], st["Ec"], OP.mult)
    def mm_row():
        ps = psB.tile([1, L], FP32, tag="ps4", name="row_ps")
        chk["row_ps"] = ps
        nc.tensor.matmul(ps, st["ones1"], st["ENQ2"], start=True, stop=True)
    def calc_drow():
        nc.scalar.activation(st["drow"], chk["row_ps"], AF.Abs)
    dmul = 2.0 if VARIANT == "base" else 1.0
    def calc_drow2():
        nc.vector.tensor_scalar(st["drow2"], st["drow"], 1.0, dmul,
                                OP.max, OP.mult)
    def calc_rrow():
        nc.vector.reciprocal(st["rrow"], st["drow2"])
    def bcast_r():
        nc.gpsimd.partition_broadcast(st["Rb"], st["rrow"])
    ops += [calc_ENQ, calc_ENQ2, mm_row, calc_drow, calc_drow2, calc_rrow,
            bcast_r]

    # -- output: h = sigmoid(o) * E * IH * (1/den); den pre-doubled for base
    def calc_EH():
        nc.vector.tensor_tensor(st["EH"], st["Ec"], chk["iht_ps"], OP.mult)
    def calc_EHR():
        nc.vector.tensor_tensor(st["EHR"], st["EH"], st["Rb"], OP.mult)
    if VARIANT == "base":
        def calc_t2():
            nc.vector.tensor_scalar(st["t2"], st["tom"][:, sl], 1.0, None, OP.add)
    else:
        def calc_t2p():
            nc.vector.tensor_scalar(st["t2p"], st["tom"][:, sl], 1.0, None, OP.add)
        def calc_t2():
            nc.vector.reciprocal(st["t2"], st["t2p"])
    def calc_h():
        nc.vector.tensor_tensor(st["houts"][:, sl], st["t2"], st["EHR"], OP.mult)
    def dma_h():
        nc.sync.dma_start(out=hout_d[:, sl], in_=st["houts"][:, sl])
    if VARIANT == "base":
        ops += [calc_EH, calc_EHR, calc_t2, calc_h, dma_h]
    else:
        ops += [calc_EH, calc_EHR, calc_t2p, calc_t2, calc_h, dma_h]

    # -- state update for next chunk
    if not last:
        def tr_K():
            ps = psB.tile([H, L], FP32, tag="ps2", name="t_ps")
            chk["kt_ps"] = ps
            nc.tensor.transpose(ps, st["k_"][:, sl], st["ident"])
        def cp_Kt():
            nc.vector.tensor_copy(st["Kts"], chk["kt_ps"])
        def mm_Cd():
            ps = psB.tile([H, H], FP32, tag="ps3", name="cd_ps")
            chk["cd_ps"] = ps
            nc.tensor.matmul(ps, st["PVt"], st["Kts"], start=True, stop=True)
        ops += [tr_K, cp_Kt, mm_Cd]
        if ci == 0:
            def upd_Cs():
                nc.vector.tensor_copy(st["Cs"], chk["cd_ps"])
        else:
            def upd_Cs():
                nc.vector.tensor_tensor(st["Cs"], st["CsS"], chk["cd_ps"], OP.add)
        ops.append(upd_Cs)

    return ops


def _build_body(nc, tc, dram):
    from contextlib import ExitStack

    with ExitStack() as ctx:
        const = ctx.enter_context(tc.tile_pool(name="const", bufs=1))
        psG = ctx.enter_context(tc.tile_pool(name="psG", bufs=2, space="PSUM"))
        psA = ctx.enter_context(tc.tile_pool(name="psA", bufs=2, space="PSUM"))
        psB = ctx.enter_context(tc.tile_pool(name="psB", bufs=1, space="PSUM"))

        st = {}

        def sb(name, shape, dtype=FP32):
            st[name] = const.tile(shape, dtype, tag=name, name=name)
            return st[name]

        # constants / weights
        for name, shape in [
            ("xT", [I, T]), ("sWT4", [I, 4 * H]), ("sRT4", [H, 4 * H]),
            ("sb4", [H, 4]), ("WT6", [H, 6 * H]), ("b6", [H, 6]),
        ]:
            sb(name, shape)
            nc.sync.dma_start(out=st[name], in_=dram[name][:])
        ident = sb("ident", [128, 128]); make_identity(nc, ident[:, :])
        tri = sb("tri", [L, L]); make_upper_triangular(nc, tri[:, :], val=1.0, diag=True)
        sb("zerL", [128, L]); nc.vector.memset(st["zerL"], 0.0)
        sb("ones1", [128, 1]); nc.vector.memset(st["ones1"], 1.0)

        # persistent buffers
        sb("Gx", [H, T, 4])
        sb("h1", [H, T + 1]); nc.vector.memset(st["h1"][:, 0:1], 0.0)
        for name in ["q_", "k_", "v_", "it_", "ft_", "tom", "F_", "a_", "u_",
                     "Ncum", "houts"]:
            sb(name, [H, T])
        for name in ["Cs", "CsS", "Cst"]:
            sb(name, [H, H])
        # sLSTM step state
        sb("cn", [H, 2]); nc.vector.memset(st["cn"], 0.0)
        sb("z1", [H, 3]); nc.vector.memset(st["z1"][:, 2:3], 1.0)
        for name in ["eif", "iz1"]:
            sb(name, [H, 2])
        for name in ["rr", "cr", "p2", "e2", "nd", "negmu", "rs"]:
            sb(name, [H, 1])
        nc.vector.memset(st["negmu"], 0.0)
        sb("g4", [H, 4]); sb("gm", [H, 4])
        sb("rc2", [H, 2]); sb("rz2", [H, 2])
        for name in ["zp1", "num", "rz", "iz", "no1",
                     "lp", "tz", "lo", "lnn", "sd", "rd", "sacc"]:
            sb(name, [H, 1])
        nc.vector.memset(st["sacc"], 1.0)
        # mLSTM chunk scratch
        for name in ["Pc", "Ec", "PKc", "Sm", "PVa", "PVt", "IHs", "Kts",
                     "ENQ", "ENQ2", "Rb", "EH", "EHR", "t2", "t2p"]:
            sb(name, [128, L])
        for name in ["negu", "ddec", "dn"]:
            sb(name, [H, 1])
        sb("drow", [1, L]); sb("drow2", [1, L]); sb("rrow", [1, L])

        gx_af = AF.Exp if VARIANT == "expv2" else AF.Identity
        for _rep in range(REPEAT):
            if VARIANT == "expv2":
                nc.vector.memset(st["sacc"], 1.0)
            # Gx precompute: Gx[:, tt, g] = sW_g @ x_t (+ sb_g); expv2 stores
            # EGx = exp(Gx + sb) instead
            for g in range(4):
                for tt in range(T // 512):
                    ps = psA.tile([H, 512], FP32, tag="gx", name="gx_ps")
                    nc.tensor.matmul(
                        ps, st["sWT4"][:, g * H : (g + 1) * H],
                        st["xT"][:, tt * 512 : (tt + 1) * 512],
                        start=True, stop=True,
                    )
                    nc.scalar.activation(
                        st["Gx"][:, tt * 512 : (tt + 1) * 512, g], ps,
                        gx_af, bias=st["sb4"][:, g : g + 1],
                    )

            # serial loop with interleaved mLSTM chunk work
            chk = {}
            pending = []
            for t in range(T):
                _emit_slstm_step(nc, st, t, psG)
                for _ in range(ML_PACE):
                    if pending:
                        pending.pop(0)()
                if (t + 1) % L == 0 and not NO_MLSTM:
                    ci = (t + 1) // L - 1
                    pending += _mlstm_chunk_ops(nc, st, ci, psB, chk,
                                                dram["hout"])
            while pending:
                pending.pop(0)()
            if NO_MLSTM:
                # still produce hout so the I/O contract holds
                nc.sync.dma_start(out=dram["hout"][:, 0:T],
                                  in_=st["h1"][:, 1 : T + 1])


def _get_nc():
    key = ("nc", VARIANT, REPEAT, NO_MLSTM, NO_RENORM, ML_PACE, RENORM,
           GP_OFF, PACK_RECIP, PACK_CN)
    if key in _NC_CACHE:
        return _NC_CACHE[key]
    nc = bacc.Bacc("TRN2", debug=False, num_devices=B)
    dram = {}
    for name, shape in [
        ("xT", [I, T]), ("sWT4", [I, 4 * H]), ("sRT4", [H, 4 * H]),
        ("sb4", [H, 4]), ("WT6", [H, 6 * H]), ("b6", [H, 6]),
    ]:
        dram[name] = nc.declare_dram_parameter(name, shape, FP32, isOutput=False)
    dram["hout"] = nc.declare_dram_parameter("hout", [H, T], FP32, isOutput=True)
    with tile.TileContext(nc) as tc:
        _build_body(nc, tc, dram)
    nc.compile()
    _NC_CACHE[key] = nc
    return nc


def _make_runner(nc):
    """Build a jitted SPMD runner for a compiled Bacc program (replicates
    bass2jax.run_bass_via_pjrt but reuses the jitted callable across calls)."""
    import jax
    from jax.sharding import Mesh, PartitionSpec
    from jax.experimental.shard_map import shard_map
    from concourse import mybir as _mb
    from concourse.bass2jax import (
        _bass_exec_p, install_neuronx_cc_hook, partition_id_tensor,
    )

    install_neuronx_cc_hook()
    partition_name = nc.partition_id_tensor.name if nc.partition_id_tensor else None
    in_names, out_names, out_avals, zero_outs = [], [], [], []
    for alloc in nc.m.functions[0].allocations:
        if not isinstance(alloc, _mb.MemoryLocationSet):
            continue
        name = alloc.memorylocations[0].name
        if alloc.kind == "ExternalInput":
            if name != partition_name:
                in_names.append(name)
        elif alloc.kind == "ExternalOutput":
            out_names.append(name)
            shape = tuple(alloc.tensor_shape)
            dtype = _mb.dt.np(alloc.dtype)
            out_avals.append(jax.core.ShapedArray(shape, dtype))
            zero_outs.append(np.zeros(shape, dtype))
    n_params = len(in_names)
    n_outs = len(out_avals)
    param_names = list(in_names)
    in_names = in_names + out_names
    if partition_name is not None:
        in_names.append(partition_name)

    def _body(*args):
        operands = list(args)
        if partition_name is not None:
            operands.append(partition_id_tensor())
        outs = _bass_exec_p.bind(
            *operands,
            out_avals=tuple(out_avals),
            in_names=tuple(in_names),
            out_names=tuple(out_names),
            lowering_input_output_aliases=(),
            sim_require_finite=True,
            sim_require_nnan=True,
            nc=nc,
        )
        return tuple(outs)

    devices = jax.devices()[:B]
    mesh = Mesh(np.asarray(devices), ("core",))
    in_specs = (PartitionSpec("core"),) * (n_params + n_outs)
    out_specs = (PartitionSpec("core"),) * n_outs
    sharded = jax.jit(
        shard_map(_body, mesh=mesh, in_specs=in_specs, out_specs=out_specs,
                  check_rep=False),
        donate_argnums=tuple(range(n_params, n_params + n_outs)),
        keep_unused=True,
    )

    def run(in_maps):
        concat_in = [
            np.concatenate([np.asarray(m[name]) for m in in_maps], axis=0)
            for name in param_names
        ]
        concat_zeros = [
            np.zeros((B * z.shape[0], *z.shape[1:]), z.dtype) for z in zero_outs
        ]
        out_arrs = sharded(*concat_in, *concat_zeros)
        out_arrs = [np.asarray(a) for a in out_arrs]
        return [
            {name: out_arrs[i].reshape(B, *out_avals[i].shape)[c]
             for i, name in enumerate(out_names)}
            for c in range(B)
        ]

    return run


def _get_runner():
    key = ("runner", VARIANT, REPEAT)
    if key not in _NC_CACHE:
        _NC_CACHE[key] = _make_runner(_get_nc())
    return _NC_CACHE[key]


def _prep_weights(inputs):
    f32 = np.float32
    sW = np.asarray(inputs["sW"], f32); sR = np.asarray(inputs["sR"], f32)
    sb_ = np.asarray(inputs["sb"], f32)
    inv_sqrt_h = f32(1.0 / np.sqrt(H))

    sWT4 = np.ascontiguousarray(sW.T); sRT4 = np.ascontiguousarray(sR.T)
    sb4 = np.ascontiguousarray(sb_.reshape(4, H).T)
    if VARIANT == "base":
        sWT4[:, 3 * H :] *= 0.5; sRT4[:, 3 * H :] *= 0.5; sb4[:, 3] *= 0.5
    else:  # expv/acto: z-rows x2 (tanh via e^{2z}), o-rows negated (sigmoid)
        sWT4[:, 2 * H : 3 * H] *= 2.0; sRT4[:, 2 * H : 3 * H] *= 2.0
        sb4[:, 2] *= 2.0
        sWT4[:, 3 * H :] *= -1.0; sRT4[:, 3 * H :] *= -1.0; sb4[:, 3] *= -1.0
    if PACK_CN:  # gate order [z, i, f, o]
        perm = [2, 0, 1, 3]
        pc = [c for g in perm for c in range(g * H, (g + 1) * H)]
        sWT4 = np.ascontiguousarray(sWT4[:, pc])
        sRT4 = np.ascontiguousarray(sRT4[:, pc])
        sb4 = np.ascontiguousarray(sb4[:, perm])

    o_scale = f32(0.5 if VARIANT == "base" else -1.0)
    WT = {}
    bvecs = []
    for j, wn, bn in [(0, "Wq", "bq"), (1, "Wk", "bk"), (2, "Wv", "bv"),
                      (3, "Wi", "bi"), (4, "Wf", "bf"), (5, "Wo", "bo")]:
        w = np.asarray(inputs[wn], f32).T.copy()
        b = np.asarray(inputs[bn], f32).copy()
        if wn == "Wk":
            w *= inv_sqrt_h; b = b * inv_sqrt_h
        if wn == "Wo":
            w *= o_scale; b = b * o_scale
        WT[j] = w
        bvecs.append(b)
    WT6 = np.ascontiguousarray(np.concatenate([WT[j] for j in range(6)], axis=1))
    b6 = np.ascontiguousarray(np.stack(bvecs, axis=1))
    return {"sWT4": sWT4, "sRT4": sRT4, "sb4": sb4, "WT6": WT6, "b6": b6}


def kernel(**inputs):
    global LAST_RESULTS
    f32 = np.float32
    x = np.ascontiguousarray(inputs["x"], dtype=f32)
    wmap = _prep_weights(inputs)

    run = _get_runner()
    in_maps = []
    for b_ in range(B):
        m = {"xT": np.ascontiguousarray(x[b_].T)}
        m.update(wmap)
        in_maps.append(m)
    results = run(in_maps)
    LAST_RESULTS = results
    out = np.empty((B, T, H), f32)
    for b_ in range(B):
        out[b_] = results[b_]["hout"].T
    return out



# revision 35
# speedup vs baseline: 1.0315x; 1.0079x over previous
"""Trainium2 Bass kernel for a 2-layer xLSTM (sLSTM -> mLSTM).

Strategy (8 NeuronCores, data-parallel over batch, 1 batch element/core):
  - Layer 0 (sLSTM): serial scan over T=1024. State kept UNNORMALIZED
    (c,n scaled by exp(m)) so no per-step max-stabilizer chain is needed:
      cn = exp(ft)*cn + exp(it - mu)*[z, 1]
    Every 16 steps the state is rescaled by an exact power of two
    (exponent-field extraction) and the log-offset mu is folded into the
    future i-gate pre-activations (Gx patch). h = 0.5*(1+tanh(o/2))*c/n
    with the o-gate weights pre-halved (sigmoid via tanh).
  - Layer 1 (mLSTM): chunkwise-parallel formulation (chunk L=128). The
    per-channel gates factor as exp(a_s - u_t) with a = i_logit - cumsum(f),
    u = running max(0, a), so each chunk reduces to a handful of 128x128
    matmuls + cumulative scans. Work for chunk c is interleaved into the
    serial sLSTM steps of chunk c+1 where the engines are otherwise idle.

kernel(**inputs) takes the FULL inputs and returns the FULL (B,T,H) output.
"""

import numpy as np

import concourse.bacc as bacc
import concourse.tile as tile
from concourse import mybir
from concourse.masks import make_identity, make_upper_triangular

AF = mybir.ActivationFunctionType
OP = mybir.AluOpType
FP32 = mybir.dt.float32
U32 = mybir.dt.uint32

B, T, I, H = 8, 1024, 128, 128
L = 128
NCHUNK = T // L
RENORM = 32
LN2 = 0.6931471805599453

TRACE = False
LAST_RESULTS = None
_NC_CACHE = {}

# step formulation: "base" (tanh+exp, V elementwise), "expv" (exp-only, V
# elementwise, no act-table switches — the shipped default), "acto" (exp/ln
# only, everything on the ACT engine)
VARIANT = "expv"
REPEAT = 1
NO_MLSTM = False    # ablation: skip layer-1 work entirely
NO_RENORM = False   # ablation: skip renorms (numerically wrong, timing only)
ML_PACE = 1         # mLSTM pending ops drained per sLSTM step
GP_OFF = False      # expv: compute n-branch (n', no1) on gpsimd
PACK_RECIP = False  # expv: single [128,2] reciprocal for rz+rr
PACK_CN = False     # expv: gate order [Z,i,f,Oe]; iz overwrites Z so the
                    # c/n update is ONE [128,2] scalar_tensor_tensor
ACT_NUM = False     # expv: compute i*Z on the idle ACT engine (8 DVE ops)


def _emit_slstm_step(nc, st, t, psG):
    """One serial sLSTM step. h1[:, t+1] <- step(h1[:, t])."""
    gps = psG.tile([H, 4], FP32, tag="g", name="gps")
    if VARIANT == "expv2":
        # psum = sR_g @ h only; Gx folded in post-exp (EGx = exp(Gx+b))
        for g in range(4):
            nc.tensor.matmul(
                gps[:, g : g + 1],
                st["sRT4"][:, g * H : (g + 1) * H],
                st["h1"][:, t : t + 1],
                start=(g == 0),
                stop=(g == 3),
            )
    else:
        # gate pre-activations: psum = Gx[:, t, :] + sR_g @ h
        nc.tensor.matmul(gps, st["ident"], st["Gx"][:, t, :], start=True,
                         stop=False)
        for g in range(4):
            nc.tensor.matmul(
                gps[:, g : g + 1],
                st["sRT4"][:, g * H : (g + 1) * H],
                st["h1"][:, t : t + 1],
                start=False,
                stop=(g == 3),
            )
    if VARIANT == "base":
        _emit_elem_base(nc, st, t, gps)
    elif VARIANT == "expv":
        _emit_elem_expv(nc, st, t, gps)
    elif VARIANT == "expv2":
        _emit_elem_expv2(nc, st, t, gps)
    else:
        _emit_elem_acto(nc, st, t, gps)
    if (t + 1) % RENORM == 0 and not NO_RENORM:
        _emit_renorm(nc, st, t)


def _emit_elem_base(nc, st, t, gps):
    """Original: exp+tanh, elementwise on V. Weights: o-rows pre-halved."""
    # eif = exp(psum[:, i,f]); z1[:,0:2] = tanh(psum[:, z,o])
    nc.scalar.activation(st["eif"], gps[:, 0:2], AF.Exp)
    nc.scalar.activation(st["z1"][:, 0:2], gps[:, 2:4], AF.Tanh)
    # iz1 = ei * [z, 1]
    nc.vector.tensor_scalar(
        st["iz1"], st["z1"][:, 0:3:2], st["eif"][:, 0:1], None, OP.mult
    )
    # cn = ef*cn + iz1
    nc.vector.scalar_tensor_tensor(
        st["cn"], st["cn"], st["eif"][:, 1:2], st["iz1"], OP.mult, OP.add
    )
    nc.vector.reciprocal(st["rr"], st["cn"][:, 1:2])
    # cr = 0.5 * c / n
    nc.vector.tensor_scalar(
        st["cr"], st["cn"][:, 0:1], st["rr"], 0.5, OP.mult, OP.mult
    )
    # h = to*cr + cr  (= sigmoid(o)*c/n with o pre-halved)
    nc.vector.scalar_tensor_tensor(
        st["h1"][:, t + 1 : t + 2], st["z1"][:, 1:2], st["cr"], st["cr"],
        OP.mult, OP.add,
    )


def _emit_elem_expv(nc, st, t, gps):
    """Exp-only gates (no act-table switches). Weights: z-rows x2, o-rows
    negated. g4 = [i, f, Z=e^{2z}, Oe=e^{-o}]; tanh z = (Z-1)/(Z+1);
    sigmoid o = 1/(1+Oe); h = c'/(n'(1+Oe)) * ... all elementwise on V."""
    g4 = st["g4"]
    if PACK_CN:
        # gate order [Z, i, f, Oe] (weights reordered host-side); cn layout
        # [c, n]. iz overwrites the consumed Z column so [iz, i] is a
        # contiguous [128,2] pair for a single fused c/n update.
        nc.scalar.activation(g4, gps[:, 0:4], AF.Exp)
        Z_, i_, f_, Oe = (g4[:, k : k + 1] for k in range(4))
        nc.vector.tensor_scalar(st["zp1"], Z_, 1.0, None, OP.add)
        nc.vector.tensor_scalar(st["num"], Z_, i_, i_, OP.mult, OP.subtract)
        nc.vector.reciprocal(st["rz"], st["zp1"])
        nc.vector.tensor_tensor(Z_, st["num"], st["rz"], OP.mult)  # iz -> col0
        # [c,n] = f*[c,n] + [iz,i]
        nc.vector.scalar_tensor_tensor(
            st["cn"], st["cn"], f_, g4[:, 0:2], OP.mult, OP.add
        )
        n_ = st["cn"][:, 1:2]
        nc.vector.scalar_tensor_tensor(st["no1"], n_, Oe, n_, OP.mult, OP.add)
        nc.vector.reciprocal(st["rr"], st["no1"])
        nc.vector.tensor_scalar(
            st["h1"][:, t + 1 : t + 2], st["cn"][:, 0:1], st["rr"], None,
            OP.mult,
        )
        return
    nc.scalar.activation(g4, gps[:, 0:4], AF.Exp)
    i_, f_, Z_, Oe = (g4[:, k : k + 1] for k in range(4))
    c_, n_ = st["cn"][:, 0:1], st["cn"][:, 1:2]
    nb = nc.gpsimd if GP_OFF else nc.vector
    if PACK_RECIP:
        zp1, no1 = st["rc2"][:, 0:1], st["rc2"][:, 1:2]
        rz, rr = st["rz2"][:, 0:1], st["rz2"][:, 1:2]
    else:
        zp1, no1, rz, rr = st["zp1"], st["no1"], st["rz"], st["rr"]
    # branch 1: iz = i*(Z-1)/(Z+1)
    nc.vector.tensor_scalar(zp1, Z_, 1.0, None, OP.add)
    if ACT_NUM:
        # i*Z on the otherwise-idle ACT engine; DVE later does (iZ - i)*rz
        nc.scalar.activation(st["num"], Z_, AF.Identity, scale=i_)
    else:
        nc.vector.tensor_scalar(st["num"], Z_, i_, i_, OP.mult, OP.subtract)
    # branch 2: n' = f*n + i ; no1 = n'*(1+Oe) ; rd = 1/no1
    nb.scalar_tensor_tensor(n_, n_, f_, i_, OP.mult, OP.add)
    nb.scalar_tensor_tensor(no1, n_, Oe, n_, OP.mult, OP.add)
    if PACK_RECIP:
        nc.vector.reciprocal(st["rz2"], st["rc2"])
    else:
        nc.vector.reciprocal(rz, zp1)
        nc.vector.reciprocal(rr, no1)
    if ACT_NUM:
        nc.vector.tensor_scalar(st["iz"], st["num"], i_, rz,
                                OP.subtract, OP.mult)
    else:
        nc.vector.tensor_tensor(st["iz"], st["num"], rz, OP.mult)
    # c' = f*c + iz ; h = c' * rd
    nc.vector.scalar_tensor_tensor(c_, c_, f_, st["iz"], OP.mult, OP.add)
    nc.vector.tensor_scalar(
        st["h1"][:, t + 1 : t + 2], c_, rr, None, OP.mult
    )


def _emit_elem_expv2(nc, st, t, gps):
    """Like expv but Gx enters multiplicatively after the exp:
    gates = exp(R h) * EGx[:, t, :]. Saves one PE matmul per step."""
    nc.scalar.activation(st["g4"], gps[:, 0:4], AF.Exp)
    gm = st["gm"]
    nc.vector.tensor_tensor(gm, st["g4"], st["Gx"][:, t, :], OP.mult)
    i_, f_, Z_, Oe = (gm[:, k : k + 1] for k in range(4))
    c_, n_ = st["cn"][:, 0:1], st["cn"][:, 1:2]
    # independent-first emission for DVE pipelining
    nc.vector.tensor_scalar(st["zp1"], Z_, 1.0, None, OP.add)
    nc.vector.tensor_scalar(st["num"], Z_, i_, i_, OP.mult, OP.subtract)
    nc.vector.scalar_tensor_tensor(n_, n_, f_, i_, OP.mult, OP.add)
    nc.vector.reciprocal(st["rz"], st["zp1"])
    nc.vector.scalar_tensor_tensor(st["no1"], n_, Oe, n_, OP.mult, OP.add)
    nc.vector.tensor_tensor(st["iz"], st["num"], st["rz"], OP.mult)
    nc.vector.reciprocal(st["rr"], st["no1"])
    nc.vector.scalar_tensor_tensor(c_, c_, f_, st["iz"], OP.mult, OP.add)
    nc.vector.tensor_scalar(
        st["h1"][:, t + 1 : t + 2], c_, st["rr"], None, OP.mult
    )


def _emit_elem_acto(nc, st, t, gps):
    """Everything on the ACT engine (exp/ln table), zero V ops: bets on
    cross-engine sem latency being the dominant cost. Weights as expv."""
    g4 = st["g4"]
    nc.scalar.activation(g4, gps[:, 0:4], AF.Exp)
    i_, f_, Z_, Oe = (g4[:, k : k + 1] for k in range(4))
    c_, n_ = st["cn"][:, 0:1], st["cn"][:, 1:2]
    A = nc.scalar.activation
    A(st["lp"], Z_, AF.Ln, bias=1.0)                      # ln(Z+1)
    A(st["rz"], st["lp"], AF.Exp, scale=-1.0)             # 1/(Z+1)
    A(st["tz"], st["rz"], AF.Identity, bias=1.0, scale=-2.0)   # tanh z
    A(st["iz"], st["tz"], AF.Identity, scale=i_)          # i*tanh
    A(c_, f_, AF.Identity, scale=c_, bias=st["iz"])       # c' = f*c + iz
    A(n_, f_, AF.Identity, scale=n_, bias=i_)             # n' = f*n + i
    A(st["lo"], Oe, AF.Ln, bias=1.0)                      # ln(1+Oe)
    A(st["lnn"], n_, AF.Ln)                               # ln n'
    A(st["sd"], st["lo"], AF.Identity, bias=st["lnn"])    # ln n' + ln(1+Oe)
    A(st["rd"], st["sd"], AF.Exp, scale=-1.0)             # 1/(n'(1+Oe))
    A(st["h1"][:, t + 1 : t + 2], c_, AF.Identity, scale=st["rd"])


def _emit_renorm(nc, st, t):
    """Rescale cn by 2^-e2(n) exactly; fold the scale into future Gx_i
    (additively in log space for base/expv; multiplicatively for expv2)."""
    cn_u = st["cn"][:, 1:2].bitcast(U32)
    nc.vector.tensor_scalar(
        st["p2"].bitcast(U32), cn_u, 0x7F800000, None, OP.bitwise_and
    )
    nc.vector.reciprocal(st["rs"], st["p2"])
    icol = 1 if PACK_CN else 0
    if VARIANT == "expv2":
        nc.vector.tensor_scalar(st["cn"], st["cn"], st["rs"], None, OP.mult)
        nc.vector.tensor_tensor(st["sacc"], st["sacc"], st["rs"], OP.mult)
        if t + 1 < T:
            hi = min(t + 1 + RENORM, T)
            sl = st["Gx"][:, t + 1 : hi, icol : icol + 1]
            nc.vector.tensor_scalar(sl, sl, st["sacc"], None, OP.mult)
        return
    nc.vector.tensor_scalar(
        st["e2"].bitcast(U32), cn_u, 23, 0x4B000000,
        OP.logical_shift_right, OP.bitwise_or,
    )
    # negdelta = -(e_biased - 127) * ln2 ; e2 holds 2^23 + e_biased as fp32
    nc.vector.tensor_scalar(
        st["nd"], st["e2"], -8388735.0, -LN2, OP.add, OP.mult
    )
    nc.vector.tensor_tensor(st["negmu"], st["negmu"], st["nd"], OP.add)
    nc.vector.tensor_scalar(st["cn"], st["cn"], st["rs"], None, OP.mult)
    if t + 1 < T:
        hi = min(t + 1 + RENORM, T)
        sl = st["Gx"][:, t + 1 : hi, icol : icol + 1]
        nc.vector.tensor_scalar(sl, sl, st["negmu"], None, OP.add)


def _mlstm_chunk_ops(nc, st, ci, psB, chk, hout_d):
    """Return a list of closures, each emitting one instruction of mLSTM
    chunk ci. Layouts: channel on partitions ([a, t]) except where noted."""
    s0 = ci * L
    sl = slice(s0, s0 + L)
    h1sl = slice(1 + s0, 1 + s0 + L)
    last = ci == NCHUNK - 1
    ops = []

    # -- projections q,k,v,it,ft,to. base: o pre-halved + Tanh (sigmoid via
    # tanh); expv/acto: o negated + Exp (sigmoid = 1/(1+e^-o), no table switch)
    o_af = AF.Tanh if VARIANT == "base" else AF.Exp
    PROJ = [("q_", AF.Identity), ("k_", AF.Identity), ("v_", AF.Identity),
            ("it_", AF.Identity), ("ft_", AF.Identity), ("tom", o_af)]

    def mk_proj(j, name, func):
        def mm():
            ps = psB.tile([H, L], FP32, tag="ps", name="proj_ps")
            chk["proj_ps"] = ps
            nc.tensor.matmul(
                ps, st["WT6"][:, j * H : (j + 1) * H], st["h1"][:, h1sl],
                start=True, stop=True,
            )
        def cp():
            nc.scalar.activation(
                st[name][:, sl], chk["proj_ps"], func,
                bias=st["b6"][:, j : j + 1],
            )
        return [mm, cp]

    for j, (name, func) in enumerate(PROJ):
        ops += mk_proj(j, name, func)

    # -- gate scans: F = cumsum(ft); a = it - F; u = runmax(0, a)
    def scan_F():
        init = 0.0 if ci == 0 else st["F_"][:, s0 - 1 : s0]
        nc.vector.tensor_tensor_scan(
            st["F_"][:, sl], st["ft_"][:, sl], st["zerL"], init, OP.add, OP.add
        )
    def calc_a():
        nc.vector.tensor_tensor(
            st["a_"][:, sl], st["it_"][:, sl], st["F_"][:, sl], OP.subtract
        )
    def scan_u():
        init = 0.0 if ci == 0 else st["u_"][:, s0 - 1 : s0]
        nc.vector.tensor_tensor_scan(
            st["u_"][:, sl], st["a_"][:, sl], st["zerL"], init, OP.max, OP.add
        )
    ops += [scan_F, calc_a, scan_u]

    u_end = st["u_"][:, s0 + L - 1 : s0 + L]

    def calc_negu():
        nc.vector.tensor_scalar(st["negu"], u_end, -1.0, None, OP.mult)
    def calc_P():
        nc.scalar.activation(st["Pc"], st["a_"][:, sl], AF.Exp, bias=st["negu"])
    def calc_E():
        nc.scalar.activation(st["Ec"], st["u_"][:, sl], AF.Exp,
                             bias=u_end, scale=-1.0)
    ops += [calc_negu, calc_P, calc_E]

    if ci > 0:
        def calc_d():
            nc.scalar.activation(st["ddec"], st["u_"][:, s0 - 1 : s0], AF.Exp,
                                 bias=st["negu"])
        def scale_Cs():
            nc.vector.tensor_scalar(st["CsS"], st["Cs"], st["ddec"], None, OP.mult)
        def tr_Cs():
            ps = psB.tile([H, H], FP32, tag="ps2", name="cst_ps")
            chk["cst_ps"] = ps
            nc.tensor.transpose(ps, st["CsS"], st["ident"])
        def cp_Cst():
            nc.vector.tensor_copy(st["Cst"], chk["cst_ps"])
        def calc_dn():
            nc.vector.tensor_scalar(
                st["dn"], st["Ncum"][:, s0 - 1 : s0], st["ddec"], None, OP.mult
            )
        ops += [calc_d, scale_Cs, tr_Cs, cp_Cst, calc_dn]

    # -- n accumulation (per-channel cumsum of P*k with decayed carry)
    def calc_PK():
        nc.vector.tensor_tensor(st["PKc"], st["Pc"], st["k_"][:, sl], OP.mult)
    def scan_N():
        init = 0.0 if ci == 0 else st["dn"]
        nc.vector.tensor_tensor_scan(
            st["Ncum"][:, sl], st["PKc"], st["zerL"], init, OP.add, OP.add
        )
    ops += [calc_PK, scan_N]

    # -- attention-style intra-chunk matmuls
    def mm_St():
        ps = psB.tile([L, L], FP32, tag="ps2", name="st_ps")
        chk["st_ps"] = ps
        nc.tensor.matmul(ps, st["k_"][:, sl], st["q_"][:, sl],
                         start=True, stop=True)
    def mask_S():
        nc.vector.tensor_tensor(st["Sm"], chk["st_ps"], st["tri"], OP.mult)
    def calc_PV():
        nc.vector.tensor_tensor(st["PVa"], st["Pc"], st["v_"][:, sl], OP.mult)
    def tr_PV():
        ps = psB.tile([H, L], FP32, tag="ps2", name="t_ps")
        chk["pvt_ps"] = ps
        nc.tensor.transpose(ps, st["PVa"], st["ident"])
    def cp_PVt():
        nc.vector.tensor_copy(st["PVt"], chk["pvt_ps"])
    ops += [mm_St, mask_S, calc_PV, tr_PV, cp_PVt]

    def mm_IH():
        ps = psB.tile([L, H], FP32, tag="ps3", name="ih_ps")
        chk["ih_ps"] = ps
        nc.tensor.matmul(ps, st["Sm"], st["PVt"], start=True, stop=(ci == 0))
    ops.append(mm_IH)
    if ci > 0:
        def mm_carry():
            nc.tensor.matmul(chk["ih_ps"], st["q_"][:, sl], st["Cst"],
                             start=False, stop=True)
        ops.append(mm_carry)

    def cp_IH():
        nc.vector.tensor_copy(st["IHs"], chk["ih_ps"])
    def tr_IH():
        ps = psB.tile([H, L], FP32, tag="ps2", name="t_ps")
        chk["iht_ps"] = ps
        nc.tensor.transpose(ps, st["IHs"], st["ident"])
    ops += [cp_IH, tr_IH]

    # -- denominator: row = sum_a E*Ncum*q ; rec = 0.5/max(|row|, 1)
    def calc_ENQ():
        nc.vector.tensor_tensor(st["ENQ"], st["Ncum"][:, sl], st["q_"][:, sl],
                                OP.mult)
    def calc_ENQ2():
        nc.vector.tensor_tensor(st["ENQ2"], st["ENQ"], st["Ec"], OP.mult)
    def mm_row():
        ps = psB.tile([1, L], FP32, tag="ps4", name="row_ps")
        chk["row_ps"] = ps
        nc.tensor.matmul(ps, st["ones1"], st["ENQ2"], start=True, stop=True)
    def calc_drow():
        nc.scalar.activation(st["drow"], chk["row_ps"], AF.Abs)
    dmul = 2.0 if VARIANT == "base" else 1.0
    def calc_drow2():
        nc.vector.tensor_scalar(st["drow2"], st["drow"], 1.0, dmul,
                                OP.max, OP.mult)
    def calc_rrow():
        nc.vector.reciprocal(st["rrow"], st["drow2"])
    def bcast_r():
        nc.gpsimd.partition_broadcast(st["Rb"], st["rrow"])
    ops += [calc_ENQ, calc_ENQ2, mm_row, calc_drow, calc_drow2, calc_rrow,
            bcast_r]

    # -- output: h = sigmoid(o) * E * IH * (1/den); den pre-doubled for base
    def calc_EH():
        nc.vector.tensor_tensor(st["EH"], st["Ec"], chk["iht_ps"], OP.mult)
    def calc_EHR():
        nc.vector.tensor_tensor(st["EHR"], st["EH"], st["Rb"], OP.mult)
    if VARIANT == "base":
        def calc_t2():
            nc.vector.tensor_scalar(st["t2"], st["tom"][:, sl], 1.0, None, OP.add)
    else:
        def calc_t2p():
            nc.vector.tensor_scalar(st["t2p"], st["tom"][:, sl], 1.0, None, OP.add)
        def calc_t2():
            nc.vector.reciprocal(st["t2"], st["t2p"])
    def calc_h():
        nc.vector.tensor_tensor(st["houts"][:, sl], st["t2"], st["EHR"], OP.mult)
    def dma_h():
        nc.sync.dma_start(out=hout_d[:, sl], in_=st["houts"][:, sl])
    if VARIANT == "base":
        ops += [calc_EH, calc_EHR, calc_t2, calc_h, dma_h]
    else:
        ops += [calc_EH, calc_EHR, calc_t2p, calc_t2, calc_h, dma_h]

    # -- state update for next chunk
    if not last:
        def tr_K():
            ps = psB.tile([H, L], FP32, tag="ps2", name="t_ps")
            chk["kt_ps"] = ps
            nc.tensor.transpose(ps, st["k_"][:, sl], st["ident"])
        def cp_Kt():
            nc.vector.tensor_copy(st["Kts"], chk["kt_ps"])
        def mm_Cd():
            ps = psB.tile([H, H], FP32, tag="ps3", name="cd_ps")
            chk["cd_ps"] = ps
            nc.tensor.matmul(ps, st["PVt"], st["Kts"], start=True, stop=True)
        ops += [tr_K, cp_Kt, mm_Cd]
        if ci == 0:
            def upd_Cs():
                nc.vector.tensor_copy(st["Cs"], chk["cd_ps"])
        else:
            def upd_Cs():
                nc.vector.tensor_tensor(st["Cs"], st["CsS"], chk["cd_ps"], OP.add)
        ops.append(upd_Cs)

    return ops


def _build_body(nc, tc, dram):
    from contextlib import ExitStack

    with ExitStack() as ctx:
        const = ctx.enter_context(tc.tile_pool(name="const", bufs=1))
        psG = ctx.enter_context(tc.tile_pool(name="psG", bufs=2, space="PSUM"))
        psA = ctx.enter_context(tc.tile_pool(name="psA", bufs=2, space="PSUM"))
        psB = ctx.enter_context(tc.tile_pool(name="psB", bufs=1, space="PSUM"))

        st = {}

        def sb(name, shape, dtype=FP32):
            st[name] = const.tile(shape, dtype, tag=name, name=name)
            return st[name]

        # constants / weights
        for name, shape in [
            ("xT", [I, T]), ("sWT4", [I, 4 * H]), ("sRT4", [H, 4 * H]),
            ("sb4", [H, 4]), ("WT6", [H, 6 * H]), ("b6", [H, 6]),
        ]:
            sb(name, shape)
            nc.sync.dma_start(out=st[name], in_=dram[name][:])
        ident = sb("ident", [128, 128]); make_identity(nc, ident[:, :])
        tri = sb("tri", [L, L]); make_upper_triangular(nc, tri[:, :], val=1.0, diag=True)
        sb("zerL", [128, L]); nc.vector.memset(st["zerL"], 0.0)
        sb("ones1", [128, 1]); nc.vector.memset(st["ones1"], 1.0)

        # persistent buffers
        sb("Gx", [H, T, 4])
        sb("h1", [H, T + 1]); nc.vector.memset(st["h1"][:, 0:1], 0.0)
        for name in ["q_", "k_", "v_", "it_", "ft_", "tom", "F_", "a_", "u_",
                     "Ncum", "houts"]:
            sb(name, [H, T])
        for name in ["Cs", "CsS", "Cst"]:
            sb(name, [H, H])
        # sLSTM step state
        sb("cn", [H, 2]); nc.vector.memset(st["cn"], 0.0)
        sb("z1", [H, 3]); nc.vector.memset(st["z1"][:, 2:3], 1.0)
        for name in ["eif", "iz1"]:
            sb(name, [H, 2])
        for name in ["rr", "cr", "p2", "e2", "nd", "negmu", "rs"]:
            sb(name, [H, 1])
        nc.vector.memset(st["negmu"], 0.0)
        sb("g4", [H, 4]); sb("gm", [H, 4])
        sb("rc2", [H, 2]); sb("rz2", [H, 2])
        for name in ["zp1", "num", "rz", "iz", "no1",
                     "lp", "tz", "lo", "lnn", "sd", "rd", "sacc"]:
            sb(name, [H, 1])
        nc.vector.memset(st["sacc"], 1.0)
        # mLSTM chunk scratch
        for name in ["Pc", "Ec", "PKc", "Sm", "PVa", "PVt", "IHs", "Kts",
                     "ENQ", "ENQ2", "Rb", "EH", "EHR", "t2", "t2p"]:
            sb(name, [128, L])
        for name in ["negu", "ddec", "dn"]:
            sb(name, [H, 1])
        sb("drow", [1, L]); sb("drow2", [1, L]); sb("rrow", [1, L])

        gx_af = AF.Exp if VARIANT == "expv2" else AF.Identity
        for _rep in range(REPEAT):
            if VARIANT == "expv2":
                nc.vector.memset(st["sacc"], 1.0)
            # Gx precompute: Gx[:, tt, g] = sW_g @ x_t (+ sb_g); expv2 stores
            # EGx = exp(Gx + sb) instead
            for g in range(4):
                for tt in range(T // 512):
                    ps = psA.tile([H, 512], FP32, tag="gx", name="gx_ps")
                    nc.tensor.matmul(
                        ps, st["sWT4"][:, g * H : (g + 1) * H],
                        st["xT"][:, tt * 512 : (tt + 1) * 512],
                        start=True, stop=True,
                    )
                    nc.scalar.activation(
                        st["Gx"][:, tt * 512 : (tt + 1) * 512, g], ps,
                        gx_af, bias=st["sb4"][:, g : g + 1],
                    )

            # serial loop with interleaved mLSTM chunk work
            chk = {}
            pending = []
            for t in range(T):
                _emit_slstm_step(nc, st, t, psG)
                for _ in range(ML_PACE):
                    if pending:
                        pending.pop(0)()
                if (t + 1) % L == 0 and not NO_MLSTM:
                    ci = (t + 1) // L - 1
                    pending += _mlstm_chunk_ops(nc, st, ci, psB, chk,
                                                dram["hout"])
            while pending:
                pending.pop(0)()
            if NO_MLSTM:
                # still produce hout so the I/O contract holds
                nc.sync.dma_start(out=dram["hout"][:, 0:T],
                                  in_=st["h1"][:, 1 : T + 1])


def _get_nc():
    key = ("nc", VARIANT, REPEAT, NO_MLSTM, NO_RENORM, ML_PACE, RENORM,
           GP_OFF, PACK_RECIP, PACK_CN, ACT_NUM)
    if key in _NC_CACHE:
        return _NC_CACHE[key]
    nc = bacc.Bacc("TRN2", debug=False, num_devices=B)
    dram = {}
    for name, shape in [
        ("xT", [I, T]), ("sWT4", [I, 4 * H]), ("sRT4", [H, 4 * H]),
        ("sb4", [H, 4]), ("WT6", [H, 6 * H]), ("b6", [H, 6]),
    ]:
        dram[name] = nc.declare_dram_parameter(name, shape, FP32, isOutput=False)
    dram["hout"] = nc.declare_dram_parameter("hout", [H, T], FP32, isOutput=True)
    with tile.TileContext(nc) as tc:
        _build_body(nc, tc, dram)
    nc.compile()
    _NC_CACHE[key] = nc
    return nc


def _make_runner(nc):
    """Build a jitted SPMD runner for a compiled Bacc program (replicates
    bass2jax.run_bass_via_pjrt but reuses the jitted callable across calls)."""
    import jax
    from jax.sharding import Mesh, PartitionSpec
    from jax.experimental.shard_map import shard_map
    from concourse import mybir as _mb
    from concourse.bass2jax import (
        _bass_exec_p, install_neuronx_cc_hook, partition_id_tensor,
    )

    install_neuronx_cc_hook()
    partition_name = nc.partition_id_tensor.name if nc.partition_id_tensor else None
    in_names, out_names, out_avals, zero_outs = [], [], [], []
    for alloc in nc.m.functions[0].allocations:
        if not isinstance(alloc, _mb.MemoryLocationSet):
            continue
        name = alloc.memorylocations[0].name
        if alloc.kind == "ExternalInput":
            if name != partition_name:
                in_names.append(name)
        elif alloc.kind == "ExternalOutput":
            out_names.append(name)
            shape = tuple(alloc.tensor_shape)
            dtype = _mb.dt.np(alloc.dtype)
            out_avals.append(jax.core.ShapedArray(shape, dtype))
            zero_outs.append(np.zeros(shape, dtype))
    n_params = len(in_names)
    n_outs = len(out_avals)
    param_names = list(in_names)
    in_names = in_names + out_names
    if partition_name is not None:
        in_names.append(partition_name)

    def _body(*args):
        operands = list(args)
        if partition_name is not None:
            operands.append(partition_id_tensor())
        outs = _bass_exec_p.bind(
            *operands,
            out_avals=tuple(out_avals),
            in_names=tuple(in_names),
            out_names=tuple(out_names),
            lowering_input_output_aliases=(),
            sim_require_finite=True,
            sim_require_nnan=True,
            nc=nc,
        )
        return tuple(outs)

    devices = jax.devices()[:B]
    mesh = Mesh(np.asarray(devices), ("core",))
    in_specs = (PartitionSpec("core"),) * (n_params + n_outs)
    out_specs = (PartitionSpec("core"),) * n_outs
    sharded = jax.jit(
        shard_map(_body, mesh=mesh, in_specs=in_specs, out_specs=out_specs,
                  check_rep=False),
        donate_argnums=tuple(range(n_params, n_params + n_outs)),
        keep_unused=True,
    )

    def run(in_maps):
        concat_in = [
            np.concatenate([np.asarray(m[name]) for m in in_maps], axis=0)
            for name in param_names
        ]
        concat_zeros = [
            np.zeros((B * z.shape[0], *z.shape[1:]), z.dtype) for z in zero_outs
        ]
        out_arrs = sharded(*concat_in, *concat_zeros)
        out_arrs = [np.asarray(a) for a in out_arrs]
        return [
            {name: out_arrs[i].reshape(B, *out_avals[i].shape)[c]
             for i, name in enumerate(out_names)}
            for c in range(B)
        ]

    return run


def _get_runner():
    key = ("runner", VARIANT, REPEAT)
    if key not in _NC_CACHE:
        _NC_CACHE[key] = _make_runner(_get_nc())
    return _NC_CACHE[key]


def _prep_weights(inputs):
    f32 = np.float32
    sW = np.asarray(inputs["sW"], f32); sR = np.asarray(inputs["sR"], f32)
    sb_ = np.asarray(inputs["sb"], f32)
    inv_sqrt_h = f32(1.0 / np.sqrt(H))

    sWT4 = np.ascontiguousarray(sW.T); sRT4 = np.ascontiguousarray(sR.T)
    sb4 = np.ascontiguousarray(sb_.reshape(4, H).T)
    if VARIANT == "base":
        sWT4[:, 3 * H :] *= 0.5; sRT4[:, 3 * H :] *= 0.5; sb4[:, 3] *= 0.5
    else:  # expv/acto: z-rows x2 (tanh via e^{2z}), o-rows negated (sigmoid)
        sWT4[:, 2 * H : 3 * H] *= 2.0; sRT4[:, 2 * H : 3 * H] *= 2.0
        sb4[:, 2] *= 2.0
        sWT4[:, 3 * H :] *= -1.0; sRT4[:, 3 * H :] *= -1.0; sb4[:, 3] *= -1.0
    if PACK_CN:  # gate order [z, i, f, o]
        perm = [2, 0, 1, 3]
        pc = [c for g in perm for c in range(g * H, (g + 1) * H)]
        sWT4 = np.ascontiguousarray(sWT4[:, pc])
        sRT4 = np.ascontiguousarray(sRT4[:, pc])
        sb4 = np.ascontiguousarray(sb4[:, perm])

    o_scale = f32(0.5 if VARIANT == "base" else -1.0)
    WT = {}
    bvecs = []
    for j, wn, bn in [(0, "Wq", "bq"), (1, "Wk", "bk"), (2, "Wv", "bv"),
                      (3, "Wi", "bi"), (4, "Wf", "bf"), (5, "Wo", "bo")]:
        w = np.asarray(inputs[wn], f32).T.copy()
        b = np.asarray(inputs[bn], f32).copy()
        if wn == "Wk":
            w *= inv_sqrt_h; b = b * inv_sqrt_h
        if wn == "Wo":
            w *= o_scale; b = b * o_scale
        WT[j] = w
        bvecs.append(b)
    WT6 = np.ascontiguousarray(np.concatenate([WT[j] for j in range(6)], axis=1))
    b6 = np.ascontiguousarray(np.stack(bvecs, axis=1))
    return {"sWT4": sWT4, "sRT4": sRT4, "sb4": sb4, "WT6": WT6, "b6": b6}


def kernel(**inputs):
    global LAST_RESULTS
    f32 = np.float32
    x = np.ascontiguousarray(inputs["x"], dtype=f32)
    wmap = _prep_weights(inputs)

    run = _get_runner()
    in_maps = []
    for b_ in range(B):
        m = {"xT": np.ascontiguousarray(x[b_].T)}
        m.update(wmap)
        in_maps.append(m)
    results = run(in_maps)
    LAST_RESULTS = results
    out = np.empty((B, T, H), f32)
    for b_ in range(B):
        out[b_] = results[b_]["hout"].T
    return out



# revision 39
# speedup vs baseline: 1.1455x; 1.1105x over previous
"""Trainium2 Bass kernel for a 2-layer xLSTM (sLSTM -> mLSTM).

Strategy (8 NeuronCores, data-parallel over batch, 1 batch element/core):
  - Layer 0 (sLSTM): serial scan over T=1024. State kept UNNORMALIZED
    (c,n scaled by exp(m)) so no per-step max-stabilizer chain is needed:
      cn = exp(ft)*cn + exp(it - mu)*[z, 1]
    Every 16 steps the state is rescaled by an exact power of two
    (exponent-field extraction) and the log-offset mu is folded into the
    future i-gate pre-activations (Gx patch). h = 0.5*(1+tanh(o/2))*c/n
    with the o-gate weights pre-halved (sigmoid via tanh).
  - Layer 1 (mLSTM): chunkwise-parallel formulation (chunk L=128). The
    per-channel gates factor as exp(a_s - u_t) with a = i_logit - cumsum(f),
    u = running max(0, a), so each chunk reduces to a handful of 128x128
    matmuls + cumulative scans. Work for chunk c is interleaved into the
    serial sLSTM steps of chunk c+1 where the engines are otherwise idle.

kernel(**inputs) takes the FULL inputs and returns the FULL (B,T,H) output.
"""

import numpy as np

import concourse.bacc as bacc
import concourse.tile as tile
from concourse import mybir
from concourse.masks import make_identity, make_upper_triangular

AF = mybir.ActivationFunctionType
OP = mybir.AluOpType
FP32 = mybir.dt.float32
U32 = mybir.dt.uint32

B, T, I, H = 8, 1024, 128, 128
L = 128
NCHUNK = T // L
RENORM = 32
LN2 = 0.6931471805599453

TRACE = False
LAST_RESULTS = None
_NC_CACHE = {}

# step formulation: "base" (tanh+exp, V elementwise), "expv" (exp-only, V
# elementwise, no act-table switches — the shipped default), "acto" (exp/ln
# only, everything on the ACT engine)
VARIANT = "expv"
REPEAT = 1
NO_MLSTM = False    # ablation: skip layer-1 work entirely
NO_RENORM = False   # ablation: skip renorms (numerically wrong, timing only)
ML_PACE = 1         # mLSTM pending ops drained per sLSTM step
GP_OFF = False      # expv: compute n-branch (n', no1) on gpsimd
PACK_RECIP = False  # expv: single [128,2] reciprocal for rz+rr
PACK_CN = False     # expv: gate order [Z,i,f,Oe]; iz overwrites Z so the
                    # c/n update is ONE [128,2] scalar_tensor_tensor
ACT_NUM = False     # expv: compute i*Z on the idle ACT engine (8 DVE ops)
GX_OVL = True       # overlap Gx precompute for t>=512 into the scan


def _emit_slstm_step(nc, st, t, psG):
    """One serial sLSTM step. h1[:, t+1] <- step(h1[:, t])."""
    gps = psG.tile([H, 4], FP32, tag="g", name="gps")
    if VARIANT == "expv2":
        # psum = sR_g @ h only; Gx folded in post-exp (EGx = exp(Gx+b))
        for g in range(4):
            nc.tensor.matmul(
                gps[:, g : g + 1],
                st["sRT4"][:, g * H : (g + 1) * H],
                st["h1"][:, t : t + 1],
                start=(g == 0),
                stop=(g == 3),
            )
    else:
        # gate pre-activations: psum = Gx[:, t, :] + sR_g @ h
        nc.tensor.matmul(gps, st["ident"], st["Gx"][:, t, :], start=True,
                         stop=False)
        for g in range(4):
            nc.tensor.matmul(
                gps[:, g : g + 1],
                st["sRT4"][:, g * H : (g + 1) * H],
                st["h1"][:, t : t + 1],
                start=False,
                stop=(g == 3),
            )
    if VARIANT == "base":
        _emit_elem_base(nc, st, t, gps)
    elif VARIANT == "expv":
        _emit_elem_expv(nc, st, t, gps)
    elif VARIANT == "expv2":
        _emit_elem_expv2(nc, st, t, gps)
    else:
        _emit_elem_acto(nc, st, t, gps)
    if (t + 1) % RENORM == 0 and not NO_RENORM:
        _emit_renorm(nc, st, t)


def _emit_elem_base(nc, st, t, gps):
    """Original: exp+tanh, elementwise on V. Weights: o-rows pre-halved."""
    # eif = exp(psum[:, i,f]); z1[:,0:2] = tanh(psum[:, z,o])
    nc.scalar.activation(st["eif"], gps[:, 0:2], AF.Exp)
    nc.scalar.activation(st["z1"][:, 0:2], gps[:, 2:4], AF.Tanh)
    # iz1 = ei * [z, 1]
    nc.vector.tensor_scalar(
        st["iz1"], st["z1"][:, 0:3:2], st["eif"][:, 0:1], None, OP.mult
    )
    # cn = ef*cn + iz1
    nc.vector.scalar_tensor_tensor(
        st["cn"], st["cn"], st["eif"][:, 1:2], st["iz1"], OP.mult, OP.add
    )
    nc.vector.reciprocal(st["rr"], st["cn"][:, 1:2])
    # cr = 0.5 * c / n
    nc.vector.tensor_scalar(
        st["cr"], st["cn"][:, 0:1], st["rr"], 0.5, OP.mult, OP.mult
    )
    # h = to*cr + cr  (= sigmoid(o)*c/n with o pre-halved)
    nc.vector.scalar_tensor_tensor(
        st["h1"][:, t + 1 : t + 2], st["z1"][:, 1:2], st["cr"], st["cr"],
        OP.mult, OP.add,
    )


def _emit_elem_expv(nc, st, t, gps):
    """Exp-only gates (no act-table switches). Weights: z-rows x2, o-rows
    negated. g4 = [i, f, Z=e^{2z}, Oe=e^{-o}]; tanh z = (Z-1)/(Z+1);
    sigmoid o = 1/(1+Oe); h = c'/(n'(1+Oe)) * ... all elementwise on V."""
    g4 = st["g4"]
    if PACK_CN:
        # gate order [Z, i, f, Oe] (weights reordered host-side); cn layout
        # [c, n]. iz overwrites the consumed Z column so [iz, i] is a
        # contiguous [128,2] pair for a single fused c/n update.
        nc.scalar.activation(g4, gps[:, 0:4], AF.Exp)
        Z_, i_, f_, Oe = (g4[:, k : k + 1] for k in range(4))
        nc.vector.tensor_scalar(st["zp1"], Z_, 1.0, None, OP.add)
        nc.vector.tensor_scalar(st["num"], Z_, i_, i_, OP.mult, OP.subtract)
        nc.vector.reciprocal(st["rz"], st["zp1"])
        nc.vector.tensor_tensor(Z_, st["num"], st["rz"], OP.mult)  # iz -> col0
        # [c,n] = f*[c,n] + [iz,i]
        nc.vector.scalar_tensor_tensor(
            st["cn"], st["cn"], f_, g4[:, 0:2], OP.mult, OP.add
        )
        n_ = st["cn"][:, 1:2]
        nc.vector.scalar_tensor_tensor(st["no1"], n_, Oe, n_, OP.mult, OP.add)
        nc.vector.reciprocal(st["rr"], st["no1"])
        nc.vector.tensor_scalar(
            st["h1"][:, t + 1 : t + 2], st["cn"][:, 0:1], st["rr"], None,
            OP.mult,
        )
        return
    nc.scalar.activation(g4, gps[:, 0:4], AF.Exp)
    i_, f_, Z_, Oe = (g4[:, k : k + 1] for k in range(4))
    c_, n_ = st["cn"][:, 0:1], st["cn"][:, 1:2]
    nb = nc.gpsimd if GP_OFF else nc.vector
    if PACK_RECIP:
        zp1, no1 = st["rc2"][:, 0:1], st["rc2"][:, 1:2]
        rz, rr = st["rz2"][:, 0:1], st["rz2"][:, 1:2]
    else:
        zp1, no1, rz, rr = st["zp1"], st["no1"], st["rz"], st["rr"]
    # branch 1: iz = i*(Z-1)/(Z+1)
    nc.vector.tensor_scalar(zp1, Z_, 1.0, None, OP.add)
    if ACT_NUM:
        # i*Z on the otherwise-idle ACT engine; DVE later does (iZ - i)*rz
        nc.scalar.activation(st["num"], Z_, AF.Identity, scale=i_)
    else:
        nc.vector.tensor_scalar(st["num"], Z_, i_, i_, OP.mult, OP.subtract)
    # branch 2: n' = f*n + i ; no1 = n'*(1+Oe) ; rd = 1/no1
    nb.scalar_tensor_tensor(n_, n_, f_, i_, OP.mult, OP.add)
    nb.scalar_tensor_tensor(no1, n_, Oe, n_, OP.mult, OP.add)
    if PACK_RECIP:
        nc.vector.reciprocal(st["rz2"], st["rc2"])
    else:
        nc.vector.reciprocal(rz, zp1)
        nc.vector.reciprocal(rr, no1)
    if ACT_NUM:
        nc.vector.tensor_scalar(st["iz"], st["num"], i_, rz,
                                OP.subtract, OP.mult)
    else:
        nc.vector.tensor_tensor(st["iz"], st["num"], rz, OP.mult)
    # c' = f*c + iz ; h = c' * rd
    nc.vector.scalar_tensor_tensor(c_, c_, f_, st["iz"], OP.mult, OP.add)
    nc.vector.tensor_scalar(
        st["h1"][:, t + 1 : t + 2], c_, rr, None, OP.mult
    )


def _emit_elem_expv2(nc, st, t, gps):
    """Like expv but Gx enters multiplicatively after the exp:
    gates = exp(R h) * EGx[:, t, :]. Saves one PE matmul per step."""
    nc.scalar.activation(st["g4"], gps[:, 0:4], AF.Exp)
    gm = st["gm"]
    nc.vector.tensor_tensor(gm, st["g4"], st["Gx"][:, t, :], OP.mult)
    i_, f_, Z_, Oe = (gm[:, k : k + 1] for k in range(4))
    c_, n_ = st["cn"][:, 0:1], st["cn"][:, 1:2]
    # independent-first emission for DVE pipelining
    nc.vector.tensor_scalar(st["zp1"], Z_, 1.0, None, OP.add)
    nc.vector.tensor_scalar(st["num"], Z_, i_, i_, OP.mult, OP.subtract)
    nc.vector.scalar_tensor_tensor(n_, n_, f_, i_, OP.mult, OP.add)
    nc.vector.reciprocal(st["rz"], st["zp1"])
    nc.vector.scalar_tensor_tensor(st["no1"], n_, Oe, n_, OP.mult, OP.add)
    nc.vector.tensor_tensor(st["iz"], st["num"], st["rz"], OP.mult)
    nc.vector.reciprocal(st["rr"], st["no1"])
    nc.vector.scalar_tensor_tensor(c_, c_, f_, st["iz"], OP.mult, OP.add)
    nc.vector.tensor_scalar(
        st["h1"][:, t + 1 : t + 2], c_, st["rr"], None, OP.mult
    )


def _emit_elem_acto(nc, st, t, gps):
    """Everything on the ACT engine (exp/ln table), zero V ops: bets on
    cross-engine sem latency being the dominant cost. Weights as expv."""
    g4 = st["g4"]
    nc.scalar.activation(g4, gps[:, 0:4], AF.Exp)
    i_, f_, Z_, Oe = (g4[:, k : k + 1] for k in range(4))
    c_, n_ = st["cn"][:, 0:1], st["cn"][:, 1:2]
    A = nc.scalar.activation
    A(st["lp"], Z_, AF.Ln, bias=1.0)                      # ln(Z+1)
    A(st["rz"], st["lp"], AF.Exp, scale=-1.0)             # 1/(Z+1)
    A(st["tz"], st["rz"], AF.Identity, bias=1.0, scale=-2.0)   # tanh z
    A(st["iz"], st["tz"], AF.Identity, scale=i_)          # i*tanh
    A(c_, f_, AF.Identity, scale=c_, bias=st["iz"])       # c' = f*c + iz
    A(n_, f_, AF.Identity, scale=n_, bias=i_)             # n' = f*n + i
    A(st["lo"], Oe, AF.Ln, bias=1.0)                      # ln(1+Oe)
    A(st["lnn"], n_, AF.Ln)                               # ln n'
    A(st["sd"], st["lo"], AF.Identity, bias=st["lnn"])    # ln n' + ln(1+Oe)
    A(st["rd"], st["sd"], AF.Exp, scale=-1.0)             # 1/(n'(1+Oe))
    A(st["h1"][:, t + 1 : t + 2], c_, AF.Identity, scale=st["rd"])


def _emit_renorm(nc, st, t):
    """Rescale cn by 2^-e2(n) exactly; fold the scale into future Gx_i
    (additively in log space for base/expv; multiplicatively for expv2)."""
    cn_u = st["cn"][:, 1:2].bitcast(U32)
    nc.vector.tensor_scalar(
        st["p2"].bitcast(U32), cn_u, 0x7F800000, None, OP.bitwise_and
    )
    nc.vector.reciprocal(st["rs"], st["p2"])
    icol = 1 if PACK_CN else 0
    if VARIANT == "expv2":
        nc.vector.tensor_scalar(st["cn"], st["cn"], st["rs"], None, OP.mult)
        nc.vector.tensor_tensor(st["sacc"], st["sacc"], st["rs"], OP.mult)
        if t + 1 < T:
            hi = min(t + 1 + RENORM, T)
            sl = st["Gx"][:, t + 1 : hi, icol : icol + 1]
            nc.vector.tensor_scalar(sl, sl, st["sacc"], None, OP.mult)
        return
    nc.vector.tensor_scalar(
        st["e2"].bitcast(U32), cn_u, 23, 0x4B000000,
        OP.logical_shift_right, OP.bitwise_or,
    )
    # negdelta = -(e_biased - 127) * ln2 ; e2 holds 2^23 + e_biased as fp32
    nc.vector.tensor_scalar(
        st["nd"], st["e2"], -8388735.0, -LN2, OP.add, OP.mult
    )
    nc.vector.tensor_tensor(st["negmu"], st["negmu"], st["nd"], OP.add)
    nc.vector.tensor_scalar(st["cn"], st["cn"], st["rs"], None, OP.mult)
    if t + 1 < T:
        hi = min(t + 1 + RENORM, T)
        sl = st["Gx"][:, t + 1 : hi, icol : icol + 1]
        nc.vector.tensor_scalar(sl, sl, st["negmu"], None, OP.add)


def _mlstm_chunk_ops(nc, st, ci, psB, chk, hout_d):
    """Return a list of closures, each emitting one instruction of mLSTM
    chunk ci. Layouts: channel on partitions ([a, t]) except where noted."""
    s0 = ci * L
    sl = slice(s0, s0 + L)
    h1sl = slice(1 + s0, 1 + s0 + L)
    last = ci == NCHUNK - 1
    ops = []

    # -- projections q,k,v,it,ft,to. base: o pre-halved + Tanh (sigmoid via
    # tanh); expv/acto: o negated + Exp (sigmoid = 1/(1+e^-o), no table switch)
    o_af = AF.Tanh if VARIANT == "base" else AF.Exp
    PROJ = [("q_", AF.Identity), ("k_", AF.Identity), ("v_", AF.Identity),
            ("it_", AF.Identity), ("ft_", AF.Identity), ("tom", o_af)]

    def mk_proj(j, name, func):
        def mm():
            ps = psB.tile([H, L], FP32, tag="ps", name="proj_ps")
            chk["proj_ps"] = ps
            nc.tensor.matmul(
                ps, st["WT6"][:, j * H : (j + 1) * H], st["h1"][:, h1sl],
                start=True, stop=True,
            )
        def cp():
            nc.scalar.activation(
                st[name][:, sl], chk["proj_ps"], func,
                bias=st["b6"][:, j : j + 1],
            )
        return [mm, cp]

    for j, (name, func) in enumerate(PROJ):
        ops += mk_proj(j, name, func)

    # -- gate scans: F = cumsum(ft); a = it - F; u = runmax(0, a)
    def scan_F():
        init = 0.0 if ci == 0 else st["F_"][:, s0 - 1 : s0]
        nc.vector.tensor_tensor_scan(
            st["F_"][:, sl], st["ft_"][:, sl], st["zerL"], init, OP.add, OP.add
        )
    def calc_a():
        nc.vector.tensor_tensor(
            st["a_"][:, sl], st["it_"][:, sl], st["F_"][:, sl], OP.subtract
        )
    def scan_u():
        init = 0.0 if ci == 0 else st["u_"][:, s0 - 1 : s0]
        nc.vector.tensor_tensor_scan(
            st["u_"][:, sl], st["a_"][:, sl], st["zerL"], init, OP.max, OP.add
        )
    ops += [scan_F, calc_a, scan_u]

    u_end = st["u_"][:, s0 + L - 1 : s0 + L]

    def calc_negu():
        nc.vector.tensor_scalar(st["negu"], u_end, -1.0, None, OP.mult)
    def calc_P():
        nc.scalar.activation(st["Pc"], st["a_"][:, sl], AF.Exp, bias=st["negu"])
    def calc_E():
        nc.scalar.activation(st["Ec"], st["u_"][:, sl], AF.Exp,
                             bias=u_end, scale=-1.0)
    ops += [calc_negu, calc_P, calc_E]

    if ci > 0:
        def calc_d():
            nc.scalar.activation(st["ddec"], st["u_"][:, s0 - 1 : s0], AF.Exp,
                                 bias=st["negu"])
        def scale_Cs():
            nc.vector.tensor_scalar(st["CsS"], st["Cs"], st["ddec"], None, OP.mult)
        def tr_Cs():
            ps = psB.tile([H, H], FP32, tag="ps2", name="cst_ps")
            chk["cst_ps"] = ps
            nc.tensor.transpose(ps, st["CsS"], st["ident"])
        def cp_Cst():
            nc.vector.tensor_copy(st["Cst"], chk["cst_ps"])
        def calc_dn():
            nc.vector.tensor_scalar(
                st["dn"], st["Ncum"][:, s0 - 1 : s0], st["ddec"], None, OP.mult
            )
        ops += [calc_d, scale_Cs, tr_Cs, cp_Cst, calc_dn]

    # -- n accumulation (per-channel cumsum of P*k with decayed carry)
    def calc_PK():
        nc.vector.tensor_tensor(st["PKc"], st["Pc"], st["k_"][:, sl], OP.mult)
    def scan_N():
        init = 0.0 if ci == 0 else st["dn"]
        nc.vector.tensor_tensor_scan(
            st["Ncum"][:, sl], st["PKc"], st["zerL"], init, OP.add, OP.add
        )
    ops += [calc_PK, scan_N]

    # -- attention-style intra-chunk matmuls
    def mm_St():
        ps = psB.tile([L, L], FP32, tag="ps2", name="st_ps")
        chk["st_ps"] = ps
        nc.tensor.matmul(ps, st["k_"][:, sl], st["q_"][:, sl],
                         start=True, stop=True)
    def mask_S():
        nc.vector.tensor_tensor(st["Sm"], chk["st_ps"], st["tri"], OP.mult)
    def calc_PV():
        nc.vector.tensor_tensor(st["PVa"], st["Pc"], st["v_"][:, sl], OP.mult)
    def tr_PV():
        ps = psB.tile([H, L], FP32, tag="ps2", name="t_ps")
        chk["pvt_ps"] = ps
        nc.tensor.transpose(ps, st["PVa"], st["ident"])
    def cp_PVt():
        nc.vector.tensor_copy(st["PVt"], chk["pvt_ps"])
    ops += [mm_St, mask_S, calc_PV, tr_PV, cp_PVt]

    def mm_IH():
        ps = psB.tile([L, H], FP32, tag="ps3", name="ih_ps")
        chk["ih_ps"] = ps
        nc.tensor.matmul(ps, st["Sm"], st["PVt"], start=True, stop=(ci == 0))
    ops.append(mm_IH)
    if ci > 0:
        def mm_carry():
            nc.tensor.matmul(chk["ih_ps"], st["q_"][:, sl], st["Cst"],
                             start=False, stop=True)
        ops.append(mm_carry)

    def cp_IH():
        nc.vector.tensor_copy(st["IHs"], chk["ih_ps"])
    def tr_IH():
        ps = psB.tile([H, L], FP32, tag="ps2", name="t_ps")
        chk["iht_ps"] = ps
        nc.tensor.transpose(ps, st["IHs"], st["ident"])
    ops += [cp_IH, tr_IH]

    # -- denominator: row = sum_a E*Ncum*q ; rec = 0.5/max(|row|, 1)
    def calc_ENQ():
        nc.vector.tensor_tensor(st["ENQ"], st["Ncum"][:, sl], st["q_"][:, sl],
                                OP.mult)
    def calc_ENQ2():
        nc.vector.tensor_tensor(st["ENQ2"], st["ENQ"], st["Ec"], OP.mult)
    def mm_row():
        ps = psB.tile([1, L], FP32, tag="ps4", name="row_ps")
        chk["row_ps"] = ps
        nc.tensor.matmul(ps, st["ones1"], st["ENQ2"], start=True, stop=True)
    def calc_drow():
        nc.scalar.activation(st["drow"], chk["row_ps"], AF.Abs)
    dmul = 2.0 if VARIANT == "base" else 1.0
    def calc_drow2():
        nc.vector.tensor_scalar(st["drow2"], st["drow"], 1.0, dmul,
                                OP.max, OP.mult)
    def calc_rrow():
        nc.vector.reciprocal(st["rrow"], st["drow2"])
    def bcast_r():
        nc.gpsimd.partition_broadcast(st["Rb"], st["rrow"])
    ops += [calc_ENQ, calc_ENQ2, mm_row, calc_drow, calc_drow2, calc_rrow,
            bcast_r]

    # -- output: h = sigmoid(o) * E * IH * (1/den); den pre-doubled for base
    def calc_EH():
        nc.vector.tensor_tensor(st["EH"], st["Ec"], chk["iht_ps"], OP.mult)
    def calc_EHR():
        nc.vector.tensor_tensor(st["EHR"], st["EH"], st["Rb"], OP.mult)
    if VARIANT == "base":
        def calc_t2():
            nc.vector.tensor_scalar(st["t2"], st["tom"][:, sl], 1.0, None, OP.add)
    else:
        def calc_t2p():
            nc.vector.tensor_scalar(st["t2p"], st["tom"][:, sl], 1.0, None, OP.add)
        def calc_t2():
            nc.vector.reciprocal(st["t2"], st["t2p"])
    def calc_h():
        nc.vector.tensor_tensor(st["houts"][:, sl], st["t2"], st["EHR"], OP.mult)
    def dma_h():
        nc.sync.dma_start(out=hout_d[:, sl], in_=st["houts"][:, sl])
    if VARIANT == "base":
        ops += [calc_EH, calc_EHR, calc_t2, calc_h, dma_h]
    else:
        ops += [calc_EH, calc_EHR, calc_t2p, calc_t2, calc_h, dma_h]

    # -- state update for next chunk
    if not last:
        def tr_K():
            ps = psB.tile([H, L], FP32, tag="ps2", name="t_ps")
            chk["kt_ps"] = ps
            nc.tensor.transpose(ps, st["k_"][:, sl], st["ident"])
        def cp_Kt():
            nc.vector.tensor_copy(st["Kts"], chk["kt_ps"])
        def mm_Cd():
            ps = psB.tile([H, H], FP32, tag="ps3", name="cd_ps")
            chk["cd_ps"] = ps
            nc.tensor.matmul(ps, st["PVt"], st["Kts"], start=True, stop=True)
        ops += [tr_K, cp_Kt, mm_Cd]
        if ci == 0:
            def upd_Cs():
                nc.vector.tensor_copy(st["Cs"], chk["cd_ps"])
        else:
            def upd_Cs():
                nc.vector.tensor_tensor(st["Cs"], st["CsS"], chk["cd_ps"], OP.add)
        ops.append(upd_Cs)

    return ops


def _build_body(nc, tc, dram):
    from contextlib import ExitStack

    with ExitStack() as ctx:
        const = ctx.enter_context(tc.tile_pool(name="const", bufs=1))
        psG = ctx.enter_context(tc.tile_pool(name="psG", bufs=2, space="PSUM"))
        psA = ctx.enter_context(tc.tile_pool(name="psA", bufs=2, space="PSUM"))
        psB = ctx.enter_context(tc.tile_pool(name="psB", bufs=1, space="PSUM"))

        st = {}

        def sb(name, shape, dtype=FP32):
            st[name] = const.tile(shape, dtype, tag=name, name=name)
            return st[name]

        # constants / weights
        for name, shape in [
            ("xT", [I, T]), ("sWT4", [I, 4 * H]), ("sRT4", [H, 4 * H]),
            ("sb4", [H, 4]), ("WT6", [H, 6 * H]), ("b6", [H, 6]),
        ]:
            sb(name, shape)
            nc.sync.dma_start(out=st[name], in_=dram[name][:])
        ident = sb("ident", [128, 128]); make_identity(nc, ident[:, :])
        tri = sb("tri", [L, L]); make_upper_triangular(nc, tri[:, :], val=1.0, diag=True)
        sb("zerL", [128, L]); nc.vector.memset(st["zerL"], 0.0)
        sb("ones1", [128, 1]); nc.vector.memset(st["ones1"], 1.0)

        # persistent buffers
        sb("Gx", [H, T, 4])
        sb("h1", [H, T + 1]); nc.vector.memset(st["h1"][:, 0:1], 0.0)
        for name in ["q_", "k_", "v_", "it_", "ft_", "tom", "F_", "a_", "u_",
                     "Ncum", "houts"]:
            sb(name, [H, T])
        for name in ["Cs", "CsS", "Cst"]:
            sb(name, [H, H])
        # sLSTM step state
        sb("cn", [H, 2]); nc.vector.memset(st["cn"], 0.0)
        sb("z1", [H, 3]); nc.vector.memset(st["z1"][:, 2:3], 1.0)
        for name in ["eif", "iz1"]:
            sb(name, [H, 2])
        for name in ["rr", "cr", "p2", "e2", "nd", "negmu", "rs"]:
            sb(name, [H, 1])
        nc.vector.memset(st["negmu"], 0.0)
        sb("g4", [H, 4]); sb("gm", [H, 4])
        sb("rc2", [H, 2]); sb("rz2", [H, 2])
        for name in ["zp1", "num", "rz", "iz", "no1",
                     "lp", "tz", "lo", "lnn", "sd", "rd", "sacc"]:
            sb(name, [H, 1])
        nc.vector.memset(st["sacc"], 1.0)
        # mLSTM chunk scratch
        for name in ["Pc", "Ec", "PKc", "Sm", "PVa", "PVt", "IHs", "Kts",
                     "ENQ", "ENQ2", "Rb", "EH", "EHR", "t2", "t2p"]:
            sb(name, [128, L])
        for name in ["negu", "ddec", "dn"]:
            sb(name, [H, 1])
        sb("drow", [1, L]); sb("drow2", [1, L]); sb("rrow", [1, L])

        gx_af = AF.Exp if VARIANT == "expv2" else AF.Identity
        for _rep in range(REPEAT):
            if VARIANT == "expv2":
                nc.vector.memset(st["sacc"], 1.0)
            # Gx precompute: Gx[:, tt, g] = sW_g @ x_t (+ sb_g); expv2 stores
            # EGx = exp(Gx + sb) instead. With GX_OVL the tt=1 half (only
            # needed from step 512) drains into the scan's idle slots.
            gx_ps = {}

            def mk_gx(g, tt):
                def op_mm():
                    ps = psA.tile([H, 512], FP32, tag="gx", name="gx_ps")
                    gx_ps[(g, tt)] = ps
                    nc.tensor.matmul(
                        ps, st["sWT4"][:, g * H : (g + 1) * H],
                        st["xT"][:, tt * 512 : (tt + 1) * 512],
                        start=True, stop=True,
                    )
                def op_act():
                    nc.scalar.activation(
                        st["Gx"][:, tt * 512 : (tt + 1) * 512, g],
                        gx_ps[(g, tt)], gx_af, bias=st["sb4"][:, g : g + 1],
                    )
                return [op_mm, op_act]

            pending = []
            for tt in range(T // 512):
                for g in range(4):
                    ops = mk_gx(g, tt)
                    if GX_OVL and tt > 0:
                        pending += ops
                    else:
                        for op in ops:
                            op()

            # serial loop with interleaved mLSTM chunk work
            chk = {}
            for t in range(T):
                _emit_slstm_step(nc, st, t, psG)
                for _ in range(ML_PACE):
                    if pending:
                        pending.pop(0)()
                if (t + 1) % L == 0 and not NO_MLSTM:
                    ci = (t + 1) // L - 1
                    pending += _mlstm_chunk_ops(nc, st, ci, psB, chk,
                                                dram["hout"])
            while pending:
                pending.pop(0)()
            if NO_MLSTM:
                # still produce hout so the I/O contract holds
                nc.sync.dma_start(out=dram["hout"][:, 0:T],
                                  in_=st["h1"][:, 1 : T + 1])


def _get_nc():
    key = ("nc", VARIANT, REPEAT, NO_MLSTM, NO_RENORM, ML_PACE, RENORM,
           GP_OFF, PACK_RECIP, PACK_CN, ACT_NUM, GX_OVL)
    if key in _NC_CACHE:
        return _NC_CACHE[key]
    nc = bacc.Bacc("TRN2", debug=False, num_devices=B)
    dram = {}
    for name, shape in [
        ("xT", [I, T]), ("sWT4", [I, 4 * H]), ("sRT4", [H, 4 * H]),
        ("sb4", [H, 4]), ("WT6", [H, 6 * H]), ("b6", [H, 6]),
    ]:
        dram[name] = nc.declare_dram_parameter(name, shape, FP32, isOutput=False)
    dram["hout"] = nc.declare_dram_parameter("hout", [H, T], FP32, isOutput=True)
    with tile.TileContext(nc) as tc:
        _build_body(nc, tc, dram)
    nc.compile()
    _NC_CACHE[key] = nc
    return nc


def _make_runner(nc):
    """Build a jitted SPMD runner for a compiled Bacc program (replicates
    bass2jax.run_bass_via_pjrt but reuses the jitted callable across calls)."""
    import jax
    from jax.sharding import Mesh, PartitionSpec
    from jax.experimental.shard_map import shard_map
    from concourse import mybir as _mb
    from concourse.bass2jax import (
        _bass_exec_p, install_neuronx_cc_hook, partition_id_tensor,
    )

    install_neuronx_cc_hook()
    partition_name = nc.partition_id_tensor.name if nc.partition_id_tensor else None
    in_names, out_names, out_avals, zero_outs = [], [], [], []
    for alloc in nc.m.functions[0].allocations:
        if not isinstance(alloc, _mb.MemoryLocationSet):
            continue
        name = alloc.memorylocations[0].name
        if alloc.kind == "ExternalInput":
            if name != partition_name:
                in_names.append(name)
        elif alloc.kind == "ExternalOutput":
            out_names.append(name)
            shape = tuple(alloc.tensor_shape)
            dtype = _mb.dt.np(alloc.dtype)
            out_avals.append(jax.core.ShapedArray(shape, dtype))
            zero_outs.append(np.zeros(shape, dtype))
    n_params = len(in_names)
    n_outs = len(out_avals)
    param_names = list(in_names)
    in_names = in_names + out_names
    if partition_name is not None:
        in_names.append(partition_name)

    def _body(*args):
        operands = list(args)
        if partition_name is not None:
            operands.append(partition_id_tensor())
        outs = _bass_exec_p.bind(
            *operands,
            out_avals=tuple(out_avals),
            in_names=tuple(in_names),
            out_names=tuple(out_names),
            lowering_input_output_aliases=(),
            sim_require_finite=True,
            sim_require_nnan=True,
            nc=nc,
        )
        return tuple(outs)

    devices = jax.devices()[:B]
    mesh = Mesh(np.asarray(devices), ("core",))
    in_specs = (PartitionSpec("core"),) * (n_params + n_outs)
    out_specs = (PartitionSpec("core"),) * n_outs
    sharded = jax.jit(
        shard_map(_body, mesh=mesh, in_specs=in_specs, out_specs=out_specs,
                  check_rep=False),
        donate_argnums=tuple(range(n_params, n_params + n_outs)),
        keep_unused=True,
    )

    def run(in_maps):
        concat_in = [
            np.concatenate([np.asarray(m[name]) for m in in_maps], axis=0)
            for name in param_names
        ]
        concat_zeros = [
            np.zeros((B * z.shape[0], *z.shape[1:]), z.dtype) for z in zero_outs
        ]
        out_arrs = sharded(*concat_in, *concat_zeros)
        out_arrs = [np.asarray(a) for a in out_arrs]
        return [
            {name: out_arrs[i].reshape(B, *out_avals[i].shape)[c]
             for i, name in enumerate(out_names)}
            for c in range(B)
        ]

    return run


def _get_runner():
    key = ("runner", VARIANT, REPEAT)
    if key not in _NC_CACHE:
        _NC_CACHE[key] = _make_runner(_get_nc())
    return _NC_CACHE[key]


def _prep_weights(inputs):
    f32 = np.float32
    sW = np.asarray(inputs["sW"], f32); sR = np.asarray(inputs["sR"], f32)
    sb_ = np.asarray(inputs["sb"], f32)
    inv_sqrt_h = f32(1.0 / np.sqrt(H))

    sWT4 = np.ascontiguousarray(sW.T); sRT4 = np.ascontiguousarray(sR.T)
    sb4 = np.ascontiguousarray(sb_.reshape(4, H).T)
    if VARIANT == "base":
        sWT4[:, 3 * H :] *= 0.5; sRT4[:, 3 * H :] *= 0.5; sb4[:, 3] *= 0.5
    else:  # expv/acto: z-rows x2 (tanh via e^{2z}), o-rows negated (sigmoid)
        sWT4[:, 2 * H : 3 * H] *= 2.0; sRT4[:, 2 * H : 3 * H] *= 2.0
        sb4[:, 2] *= 2.0
        sWT4[:, 3 * H :] *= -1.0; sRT4[:, 3 * H :] *= -1.0; sb4[:, 3] *= -1.0
    if PACK_CN:  # gate order [z, i, f, o]
        perm = [2, 0, 1, 3]
        pc = [c for g in perm for c in range(g * H, (g + 1) * H)]
        sWT4 = np.ascontiguousarray(sWT4[:, pc])
        sRT4 = np.ascontiguousarray(sRT4[:, pc])
        sb4 = np.ascontiguousarray(sb4[:, perm])

    o_scale = f32(0.5 if VARIANT == "base" else -1.0)
    WT = {}
    bvecs = []
    for j, wn, bn in [(0, "Wq", "bq"), (1, "Wk", "bk"), (2, "Wv", "bv"),
                      (3, "Wi", "bi"), (4, "Wf", "bf"), (5, "Wo", "bo")]:
        w = np.asarray(inputs[wn], f32).T.copy()
        b = np.asarray(inputs[bn], f32).copy()
        if wn == "Wk":
            w *= inv_sqrt_h; b = b * inv_sqrt_h
        if wn == "Wo":
            w *= o_scale; b = b * o_scale
        WT[j] = w
        bvecs.append(b)
    WT6 = np.ascontiguousarray(np.concatenate([WT[j] for j in range(6)], axis=1))
    b6 = np.ascontiguousarray(np.stack(bvecs, axis=1))
    return {"sWT4": sWT4, "sRT4": sRT4, "sb4": sb4, "WT6": WT6, "b6": b6}


def kernel(**inputs):
    global LAST_RESULTS
    f32 = np.float32
    x = np.ascontiguousarray(inputs["x"], dtype=f32)
    wmap = _prep_weights(inputs)

    run = _get_runner()
    in_maps = []
    for b_ in range(B):
        m = {"xT": np.ascontiguousarray(x[b_].T)}
        m.update(wmap)
        in_maps.append(m)
    results = run(in_maps)
    LAST_RESULTS = results
    out = np.empty((B, T, H), f32)
    for b_ in range(B):
        out[b_] = results[b_]["hout"].T
    return out

